# revision 1
# baseline (speedup 1.0000x reference)
"""GNN message-passing kernel for Trainium2 (8 NeuronCores, SPMD).

Computation (see reference):
  h1 = tanh(segsum(x[src] -> dst) @ W1 + b1)        [uses A(xW) = (Ax)W]
  h2 = tanh(segsum(h1[src] -> dst) @ W2 + b2)
  ht = logmap0(proj(h2))  (rowwise scale)
  pooled = segment mean over seg_ids, then expmap0/proj (host epilogue)

Sharding: nodes split contiguously over cores (dst-shard). Each core owns
SHARD nodes, processes the edges whose dst is in its shard.  The spmm is a
one-hot matmul: for each 128-edge tile, S^T[e,slot] = (dstslot[e]==slot)
(DVE is_equal vs iota), stationary lhsT=S^T, moving rhs = gathered rows.
Gather via gpsimd.dma_gather with int16 indices (tables chunked to 32768
rows).  The only cross-core exchange is one AllGather of h1 (bf16).
"""

import math
from contextlib import ExitStack

import numpy as np
import ml_dtypes

import concourse.bass as bass
import concourse.tile as tile
import concourse.bacc as bacc
from concourse import mybir

BF16 = mybir.dt.bfloat16
F32 = mybir.dt.float32
I16 = mybir.dt.int16
AF = mybir.ActivationFunctionType
ALU = mybir.AluOpType

MAXNORM = 1.0 - 1e-5
MIN_SS = 1e-15

SUB = 1024          # gather indices per dma_gather call (descriptor ring limit)
GRP = 4             # dst blocks (of 128 nodes) per PSUM group


class Cfg:
    def __init__(self, n_nodes, in_dim, hid, n_seg, n_cores):
        self.N = n_nodes
        self.IN = in_dim
        self.HID = hid
        self.NSEG = n_seg
        self.NC = n_cores
        self.SHARD = n_nodes // n_cores
        assert self.SHARD % 128 == 0
        self.NBLK = self.SHARD // 128
        assert self.NBLK % GRP == 0
        self.NGRP = self.NBLK // GRP
        self.CH = min(32768, n_nodes)
        assert n_nodes % self.CH == 0
        self.NCHUNK = n_nodes // self.CH
        self.NSEGCH = (n_seg + 127) // 128


def host_prep(cfg, src, dst):
    """Build SPMD-uniform edge tiling + per-core index/slot arrays.

    Returns (ntiles[NGRP,NCHUNK,GRP], per-core list of dicts with
    idx16 [128, TOT/16] int16 and dstslot [128, NTILES] float arrays).
    """
    NC, SHARD, CH = cfg.NC, cfg.SHARD, cfg.CH
    src = np.asarray(src).astype(np.int64)
    dst = np.asarray(dst).astype(np.int64)

    core = dst // SHARD
    blk = (dst % SHARD) // 128          # block within core [0, NBLK)
    slot = dst % 128
    chunk = src // CH
    idx = src % CH

    # counts[c, g, k, b]
    counts = np.zeros((NC, cfg.NGRP, cfg.NCHUNK, GRP), dtype=np.int64)
    g_all = blk // GRP
    b_all = blk % GRP
    np.add.at(counts, (core, g_all, chunk, b_all), 1)

    mx = counts.max(axis=0)
    ntiles = (mx + 127) // 128
    # ensure every block has >= 1 tile in chunk 0 (so PSUM gets a start write)
    empty = ntiles.sum(axis=1) == 0      # [NGRP, GRP]
    ntiles[:, 0, :][empty] = 1

    NTILES = int(ntiles.sum())
    TOT = NTILES * 128

    per_core = []
    # canonical ordering: g, k, b, then edges of that cell (+pad)
    order = np.lexsort((idx, b_all, chunk, g_all, core))
    # cell boundaries per core
    for c in range(NC):
        idx16 = np.zeros(TOT, dtype=np.int16)
        slots = np.full(TOT, -1.0, dtype=np.float32)
        sel = order[core[order] == c]
        csrc_idx = idx[sel]
        cslot = slot[sel]
        cg = g_all[sel]
        ck = chunk[sel]
        cb = b_all[sel]
        # counts per cell for this core
        ccnt = counts[c]
        pos = 0      # position in canonical padded stream
        ep = 0       # position in sel
        for g in range(cfg.NGRP):
            for k in range(cfg.NCHUNK):
                for b in range(GRP):
                    n = int(ccnt[g, k, b])
                    cap = int(ntiles[g, k, b]) * 128
                    if n > 0:
                        idx16[pos:pos + n] = csrc_idx[ep:ep + n]
                        slots[pos:pos + n] = cslot[ep:ep + n]
                        # sanity
                        assert np.all(cg[ep:ep + n] == g)
                        assert np.all(ck[ep:ep + n] == k)
                        assert np.all(cb[ep:ep + n] == b)
                        ep += n
                    pos += cap
        assert ep == len(sel)
        # wrap idx: i -> [i%16, i//16], replicate x8 partitions
        iw = idx16.reshape(-1, 16).T            # [16, TOT/16]
        iw = np.tile(iw, (8, 1)).copy()         # [128, TOT/16]
        # dstslot tile-major: [128 (edge in tile), NTILES]
        sl = slots.reshape(NTILES, 128).T.copy()
        per_core.append({"idx16": iw.astype(np.int16),
                         "dstslot": sl.astype(np.float32)})
    return ntiles, per_core


def _mm_schedule(cfg, ntiles):
    """Per (g): list over chunks of list of (tile_global_col, block b, start, stop)."""
    sched = []
    tcol = 0
    for g in range(cfg.NGRP):
        # first/last tile of each block across chunks
        tot_b = ntiles[g].sum(axis=0)   # [GRP]
        seen_b = np.zeros(GRP, dtype=np.int64)
        chunks = []
        for k in range(cfg.NCHUNK):
            tiles_k = []
            for b in range(GRP):
                for _ in range(int(ntiles[g, k, b])):
                    start = seen_b[b] == 0
                    stop = seen_b[b] == tot_b[b] - 1
                    tiles_k.append((tcol, b, bool(start), bool(stop)))
                    seen_b[b] += 1
                    tcol += 1
            chunks.append(tiles_k)
        sched.append(chunks)
    return sched


def build(cfg, ntiles, n_reps=1, debug_taps=False):
    """Build the Bass program. Returns nc."""
    N, IN, HID = cfg.N, cfg.IN, cfg.HID
    NTILES = int(ntiles.sum())
    TOT = NTILES * 128
    sched = _mm_schedule(cfg, ntiles)

    nc = bacc.Bacc("TRN2", target_bir_lowering=False)

    x_d = nc.dram_tensor("x_bf16", [N, IN], BF16, kind="ExternalInput")
    idx_d = nc.dram_tensor("idx16", [128, TOT // 16], I16, kind="ExternalInput")
    slot_d = nc.dram_tensor("dstslot", [128, NTILES], F32, kind="ExternalInput")
    segid_d = nc.dram_tensor("segid", [128, cfg.NBLK], F32, kind="ExternalInput")
    iota_d = nc.dram_tensor("iota128", [128, 128], BF16, kind="ExternalInput")
    iotas_d = nc.dram_tensor("iota_seg", [128, cfg.NSEGCH * 128], F32, kind="ExternalInput")
    ident_d = nc.dram_tensor("ident", [128, 128], BF16, kind="ExternalInput")
    w1_d = nc.dram_tensor("W1", [IN, HID], BF16, kind="ExternalInput")
    w2_d = nc.dram_tensor("W2", [HID, HID], BF16, kind="ExternalInput")
    b1_d = nc.dram_tensor("b1rep", [128, HID], F32, kind="ExternalInput")
    b2_d = nc.dram_tensor("b2rep", [128, HID], F32, kind="ExternalInput")

    h1_shard = nc.dram_tensor("h1_shard", [cfg.SHARD, HID], BF16)
    h1_full = nc.dram_tensor("h1_full", [N, HID], BF16, addr_space="Shared")
    out_d = nc.dram_tensor("pooled", [cfg.NSEGCH * 128, HID + 1], F32,
                           kind="ExternalOutput")
    if debug_taps:
        dbg_h1 = nc.dram_tensor("dbg_h1", [cfg.SHARD, HID], F32, kind="ExternalOutput")
        dbg_h2 = nc.dram_tensor("dbg_h2", [128, cfg.NBLK * HID], F32, kind="ExternalOutput")
        dbg_sc = nc.dram_tensor("dbg_sc", [128, 2 * cfg.NBLK], F32, kind="ExternalOutput")

    KIN = IN // 128   # k-chunks for W1 (2)

    with tile.TileContext(nc) as tc, ExitStack() as ctx:
        const = ctx.enter_context(tc.tile_pool(name="const", bufs=1))
        idxp = ctx.enter_context(tc.tile_pool(name="idxp", bufs=4))
        slotp = ctx.enter_context(tc.tile_pool(name="slotp", bufs=3))
        ebufp = ctx.enter_context(tc.tile_pool(name="ebufp", bufs=4))
        sp = ctx.enter_context(tc.tile_pool(name="sp", bufs=4))
        flshp = ctx.enter_context(tc.tile_pool(name="flshp", bufs=3))
        xtp = ctx.enter_context(tc.tile_pool(name="xtp", bufs=4))
        hp = ctx.enter_context(tc.tile_pool(name="hp", bufs=3))
        h2allp = ctx.enter_context(tc.tile_pool(name="h2allp", bufs=1))
        normp = ctx.enter_context(tc.tile_pool(name="normp", bufs=1))
        htp = ctx.enter_context(tc.tile_pool(name="htp", bufs=3))

        ctx_spmm = ctx.enter_context(ExitStack())
        ps_acc = ctx_spmm.enter_context(tc.tile_pool(name="ps_acc", bufs=4, space="PSUM"))
        ps_tr = ctx_spmm.enter_context(tc.tile_pool(name="ps_tr", bufs=1, space="PSUM"))
        ps_h = ctx_spmm.enter_context(tc.tile_pool(name="ps_h", bufs=2, space="PSUM"))

        # ---- constants ----
        iota128 = const.tile([128, 128], BF16)
        nc.sync.dma_start(iota128[:], iota_d[:])
        iotaseg = const.tile([128, cfg.NSEGCH * 128], F32)
        nc.sync.dma_start(iotaseg[:], iotas_d[:])
        ident = const.tile([128, 128], BF16)
        nc.sync.dma_start(ident[:], ident_d[:])
        segid = const.tile([128, cfg.NBLK], F32)
        nc.sync.dma_start(segid[:], segid_d[:])
        w1_sb = [const.tile([128, HID], BF16, tag=f"w1_{k}", name=f"w1_{k}")
                 for k in range(KIN)]
        for k in range(KIN):
            nc.sync.dma_start(w1_sb[k][:], w1_d[k * 128:(k + 1) * 128, :])
        w2_sb = const.tile([128, HID], BF16)
        nc.sync.dma_start(w2_sb[:], w2_d[:])
        b1_sb = const.tile([128, HID], F32)
        nc.sync.dma_start(b1_sb[:], b1_d[:])
        b2_sb = const.tile([128, HID], F32)
        nc.sync.dma_start(b2_sb[:], b2_d[:])

        h2_all = h2allp.tile([128, cfg.NBLK * HID], F32)
        norms2 = normp.tile([128, cfg.NBLK], F32)
        scale = normp.tile([128, cfg.NBLK], F32)
        na = normp.tile([128, cfg.NBLK], F32)
        nb_t = normp.tile([128, cfg.NBLK], F32)

        def spmm_layer(layer, table_ap, feat, out_block):
            """One spmm layer.  table_ap: DRAM [N, feat] gather table.
            out_block(g, b, agg_ps) consumes the accumulated [128(slot),
            feat] PSUM tile for global block nb=g*GRP+b.  One PSUM bank
            per block: start=True clears has_written bank-wide on HW, so
            accumulation groups must not share a bank."""
            for g in range(cfg.NGRP):
                accs = [ps_acc.tile([128, IN], F32, tag="acc", name=f"acc{b}")
                        for b in range(GRP)]

                def acc_slice(b):
                    return accs[b][:, :feat]

                for k in range(cfg.NCHUNK):
                    tiles_k = sched[g][k]
                    if not tiles_k:
                        continue
                    tbl = table_ap[k * cfg.CH:(k + 1) * cfg.CH, :]
                    # subcalls of <= SUB indices
                    for s0 in range(0, len(tiles_k), SUB // 128):
                        stiles = tiles_k[s0:s0 + SUB // 128]
                        nidx = len(stiles) * 128
                        col0 = stiles[0][0]  # global tile col
                        it = idxp.tile([128, SUB // 16], I16, tag="it")
                        nc.sync.dma_start(
                            it[:, :nidx // 16],
                            idx_d[:, col0 * 8:col0 * 8 + nidx // 16])
                        st = slotp.tile([128, SUB // 128], F32, tag="st")
                        nc.sync.dma_start(
                            st[:, :len(stiles)],
                            slot_d[:, col0:col0 + len(stiles)])
                        eb = ebufp.tile([128, (SUB // 128) * feat], BF16,
                                        tag=f"eb{layer}")
                        nc.gpsimd.dma_gather(
                            out_ap=eb[:, :len(stiles) * feat].rearrange(
                                "p (n f) -> p n f", f=feat),
                            in_ap=tbl,
                            idxs_ap=it[:, :nidx // 16],
                            num_idxs=nidx,
                            num_idxs_reg=nidx,
                            elem_size=feat,
                        )
                        for j, (tcol, b, st_f, sp_f) in enumerate(stiles):
                            s_t = sp.tile([128, 128], BF16, tag="s_t")
                            nc.vector.tensor_scalar(
                                s_t[:], iota128[:], st[:, j:j + 1], None,
                                ALU.is_equal)
                            nc.tensor.matmul(
                                acc_slice(b),
                                s_t[:],
                                eb[:, j * feat:(j + 1) * feat],
                                start=st_f, stop=sp_f,
                            )
                for b in range(GRP):
                    out_block(g, b, acc_slice(b))

        def l1_block(g, b, agg_ps):
            nb = g * GRP + b
            # copy PSUM f32 -> SBUF bf16
            ax = flshp.tile([128, IN], BF16, tag="ax1")
            nc.scalar.activation(ax[:], agg_ps, AF.Copy)
            h_ps = ps_h.tile([128, HID], F32, tag="hps", name="h_ps")
            for h in range(KIN):
                t_ps = ps_tr.tile([128, 128], BF16, tag="tps")
                nc.tensor.transpose(t_ps[:], ax[:, h * 128:(h + 1) * 128], ident[:])
                xt = xtp.tile([128, 128], BF16, tag="xt")
                nc.scalar.activation(xt[:], t_ps[:], AF.Copy)
                nc.tensor.matmul(h_ps[:], xt[:], w1_sb[h][:],
                                 start=(h == 0), stop=(h == KIN - 1))
            htmp = hp.tile([128, HID], F32, tag="htmp")
            nc.vector.tensor_add(htmp[:], h_ps[:], b1_sb[:])
            h1b = hp.tile([128, HID], BF16, tag="h1b")
            nc.scalar.activation(h1b[:], htmp[:], AF.Tanh)
            nc.sync.dma_start(h1_shard[nb * 128:(nb + 1) * 128, :], h1b[:])
            if debug_taps:
                h1f = hp.tile([128, HID], F32, tag="h1f")
                nc.scalar.activation(h1f[:], htmp[:], AF.Tanh)
                nc.sync.dma_start(dbg_h1[nb * 128:(nb + 1) * 128, :], h1f[:])

        def l2_block(g, b, agg_ps):
            nb = g * GRP + b
            a2 = flshp.tile([128, HID], BF16, tag="a22")
            nc.scalar.activation(a2[:], agg_ps, AF.Copy)
            t_ps = ps_tr.tile([128, 128], BF16, tag="tps")
            nc.tensor.transpose(t_ps[:], a2[:], ident[:])
            a2t = xtp.tile([128, 128], BF16, tag="xt")
            nc.scalar.activation(a2t[:], t_ps[:], AF.Copy)
            h_ps = ps_h.tile([128, HID], F32, tag="hps", name="h_ps")
            nc.tensor.matmul(h_ps[:], a2t[:], w2_sb[:], start=True, stop=True)
            htmp = hp.tile([128, HID], F32, tag="htmp")
            nc.vector.tensor_add(htmp[:], h_ps[:], b2_sb[:])
            nc.scalar.activation(h2_all[:, nb * HID:(nb + 1) * HID], htmp[:],
                                 AF.Tanh)

        # ---------------- layer 1 ----------------
        spmm_layer(1, x_d, IN, l1_block)

        # ---------------- exchange ----------------
        nc.gpsimd.collective_compute(
            "AllGather",
            ALU.bypass,
            ins=[h1_shard.ap().opt()],
            outs=[h1_full.ap().opt()],
            replica_groups=[list(range(cfg.NC))],
        )

        # ---------------- layer 2 ----------------
        spmm_layer(2, h1_full, HID, l2_block)

        # ---------------- norms + logmap scale ----------------
        for nbk in range(cfg.NBLK):
            h2b = h2_all[:, nbk * HID:(nbk + 1) * HID]
            sq = htp.tile([128, HID], F32, tag="sq")
            nc.vector.tensor_mul(sq[:], h2b, h2b)
            nc.vector.tensor_reduce(norms2[:, nbk:nbk + 1], sq[:],
                                    mybir.AxisListType.X, ALU.add)
        # norm = sqrt(max(ss, MIN_SS)); nclip = min(norm, MAXNORM)
        nc.vector.tensor_scalar_max(na[:], norms2[:], MIN_SS)
        nc.scalar.activation(nb_t[:], na[:], AF.Sqrt)        # nb_t = norm
        nc.vector.tensor_scalar_min(na[:], nb_t[:], MAXNORM)  # na = nclip
        # artanh(nclip) = 0.5*ln((1+n)/(1-n)); scale = artanh/norm
        one_m = normp.tile([128, cfg.NBLK], F32)
        nc.vector.tensor_scalar(one_m[:], na[:], -1.0, 1.0, ALU.mult, ALU.add)
        one_p = normp.tile([128, cfg.NBLK], F32)
        nc.vector.tensor_scalar_add(one_p[:], na[:], 1.0)
        rcp = normp.tile([128, cfg.NBLK], F32)
        nc.vector.reciprocal(rcp[:], one_m[:])
        rat = normp.tile([128, cfg.NBLK], F32)
        nc.vector.tensor_mul(rat[:], one_p[:], rcp[:])
        lg = normp.tile([128, cfg.NBLK], F32)
        nc.scalar.activation(lg[:], rat[:], AF.Ln)
        nc.vector.tensor_scalar_mul(lg[:], lg[:], 0.5)
        rcpn = normp.tile([128, cfg.NBLK], F32)
        nc.vector.reciprocal(rcpn[:], nb_t[:])
        nc.vector.tensor_mul(scale[:], lg[:], rcpn[:])

        if debug_taps:
            nc.sync.dma_start(dbg_h2[:], h2_all[:])
            nc.sync.dma_start(dbg_sc[:, :cfg.NBLK], norms2[:])
            nc.sync.dma_start(dbg_sc[:, cfg.NBLK:], scale[:])
        # ---------------- pooling ----------------
        ctx_spmm.close()
        ps_pool = ctx.enter_context(
            tc.tile_pool(name="ps_pool", bufs=max(cfg.NSEGCH, 1), space="PSUM"))
        pool_ps = [ps_pool.tile([128, HID + 1], F32, tag="pool", name=f"pool{sc}")
                   for sc in range(cfg.NSEGCH)]
        for nbk in range(cfg.NBLK):
            h2b = h2_all[:, nbk * HID:(nbk + 1) * HID]
            ht = htp.tile([128, HID + 1], BF16, tag="ht")
            nc.vector.tensor_scalar(ht[:, :HID], h2b, scale[:, nbk:nbk + 1],
                                    None, ALU.mult)
            nc.vector.memset(ht[:, HID:HID + 1], 1.0)
            for sc in range(cfg.NSEGCH):
                sg = sp.tile([128, 128], BF16, tag="sg")
                nc.vector.tensor_scalar(
                    sg[:], iotaseg[:, sc * 128:(sc + 1) * 128],
                    segid[:, nbk:nbk + 1], None, ALU.is_equal)
                nc.tensor.matmul(
                    pool_ps[sc][:], sg[:], ht[:],
                    start=(nbk == 0), stop=(nbk == cfg.NBLK - 1))
        for sc in range(cfg.NSEGCH):
            po = htp.tile([128, HID + 1], F32, tag="po")
            nc.vector.tensor_copy(po[:], pool_ps[sc][:])
            nc.sync.dma_start(out_d[sc * 128:(sc + 1) * 128, :], po[:])

    nc.compile()
    return nc


def host_inputs(cfg, x, seg_ids, W1, b1, W2, b2, per_core):
    """Per-core in_maps for run_bass_kernel_spmd."""
    N, IN, HID = cfg.N, cfg.IN, cfg.HID
    x_bf16 = np.ascontiguousarray(x.astype(ml_dtypes.bfloat16))
    iota128 = np.tile(np.arange(128, dtype=np.float32), (128, 1)).astype(ml_dtypes.bfloat16)
    iotaseg = np.tile(np.arange(cfg.NSEGCH * 128, dtype=np.float32), (128, 1))
    ident = np.eye(128, dtype=np.float32).astype(ml_dtypes.bfloat16)
    w1 = np.ascontiguousarray(W1.astype(ml_dtypes.bfloat16))
    w2 = np.ascontiguousarray(W2.astype(ml_dtypes.bfloat16))
    b1r = np.tile(np.asarray(b1, np.float32), (128, 1))
    b2r = np.tile(np.asarray(b2, np.float32), (128, 1))
    seg = np.asarray(seg_ids, np.float32)
    maps = []
    for c in range(cfg.NC):
        segc = seg[c * cfg.SHARD:(c + 1) * cfg.SHARD].reshape(cfg.NBLK, 128).T
        maps.append({
            "x_bf16": x_bf16,
            "idx16": per_core[c]["idx16"],
            "dstslot": per_core[c]["dstslot"],
            "segid": np.ascontiguousarray(segc),
            "iota128": iota128,
            "iota_seg": np.ascontiguousarray(iotaseg.astype(np.float32)),
            "ident": ident,
            "W1": w1,
            "W2": w2,
            "b1rep": b1r,
            "b2rep": b2r,
        })
    return maps


def host_epilogue(cfg, partials, batch_size, max_comments):
    """partials: list of per-core [NSEGCH*128, HID+1] f32."""
    acc = np.zeros_like(partials[0], dtype=np.float64)
    for p in partials:
        acc += p.astype(np.float64)
    acc = acc.astype(np.float32)
    nseg = cfg.NSEG
    sums = acc[:nseg, :cfg.HID]
    counts = acc[:nseg, cfg.HID]
    agg = sums / np.maximum(counts, 1.0)[:, None]
    # expmap0 then proj
    ss = np.maximum(np.sum(agg * agg, axis=1), MIN_SS).astype(np.float32)
    norm = np.sqrt(ss)
    y = agg * (np.tanh(norm) / norm)[:, None]
    ssy = np.maximum(np.sum(y * y, axis=1), MIN_SS).astype(np.float32)
    ny = np.sqrt(ssy)
    f = np.where(ny > MAXNORM, MAXNORM / ny, 1.0).astype(np.float32)
    y = y * f[:, None]
    return y.reshape(int(batch_size), int(max_comments), cfg.HID)


# ---------------- numpy reference (for arbitrary sizes) ----------------

def np_reference(x, src, dst, seg_ids, W1, b1, W2, b2, batch_size, max_comments):
    n = x.shape[0]

    def seg_sum(vals, ids, nseg):
        out = np.zeros((nseg, vals.shape[1]), np.float32)
        np.add.at(out, ids, vals)
        return out

    def rownorm(v):
        return np.sqrt(np.maximum(np.sum(v * v, axis=1, keepdims=True), MIN_SS))

    def proj(v):
        nn = rownorm(v)
        return np.where(nn > MAXNORM, v / nn * MAXNORM, v)

    def logmap0(v):
        nn = rownorm(v)
        arg = np.minimum(nn, 1 - 1e-7)
        return v * np.arctanh(arg) / nn

    def expmap0(v):
        nn = rownorm(v)
        return v * np.tanh(nn) / nn

    h = np.tanh(seg_sum(x[src] @ W1, dst, n) + b1)
    h = np.tanh(seg_sum(h[src] @ W2, dst, n) + b2)
    h = logmap0(proj(h))
    nseg = int(batch_size) * int(max_comments)
    sums = seg_sum(h, seg_ids, nseg)
    counts = np.zeros(nseg, np.float32)
    np.add.at(counts, seg_ids, 1.0)
    agg = sums / np.maximum(counts, 1.0)[:, None]
    agg = proj(expmap0(agg))
    return agg.reshape(int(batch_size), int(max_comments), -1)


# ====================================================================
# Harness entry point: kernel(**inputs) -> np.ndarray
# ====================================================================

_CACHE = {}


def kernel(x, src, dst, seg_ids, W1, b1, W2, b2, batch_size, max_comments):
    """Full-input GNN ComEnc kernel on 8 Trainium2 NeuronCores.

    Accepts the unsharded inputs of reference.setup_inputs() and returns
    the full (batch, max_comments, HID) float32 output.
    """
    from concourse.bass_utils import run_bass_kernel_spmd

    x = np.asarray(x, dtype=np.float32)
    src = np.asarray(src).astype(np.int64)
    dst = np.asarray(dst).astype(np.int64)
    seg_ids = np.asarray(seg_ids).astype(np.int64)
    W1 = np.asarray(W1, dtype=np.float32)
    b1 = np.asarray(b1, dtype=np.float32)
    W2 = np.asarray(W2, dtype=np.float32)
    b2 = np.asarray(b2, dtype=np.float32)
    bs = int(np.asarray(batch_size))
    mc = int(np.asarray(max_comments))

    n_nodes, in_dim = x.shape
    hid = W1.shape[1]
    nseg = bs * mc
    n_cores = 8

    cfg = Cfg(n_nodes, in_dim, hid, nseg, n_cores)
    ntiles, per_core = host_prep(cfg, src, dst)

    key = (n_nodes, in_dim, hid, nseg, ntiles.tobytes())
    if key in _CACHE:
        nc = _CACHE[key]
    else:
        nc = build(cfg, ntiles)
        _CACHE.clear()
        _CACHE[key] = nc

    maps = host_inputs(cfg, x, seg_ids, W1, b1, W2, b2, per_core)
    res = run_bass_kernel_spmd(nc, maps, core_ids=list(range(n_cores)))
    partials = [r["pooled"] for r in res.results]
    out = host_epilogue(cfg, partials, bs, mc)
    return np.ascontiguousarray(out.astype(np.float32))



# revision 12
# speedup vs baseline: 1.3250x; 1.3250x over previous
"""GNN message-passing kernel for Trainium2 (8 NeuronCores, SPMD).

Computation (see reference):
  h1 = tanh(segsum(x[src] -> dst) @ W1 + b1)        [uses A(xW) = (Ax)W]
  h2 = tanh(segsum(h1[src] -> dst) @ W2 + b2)
  ht = logmap0(proj(h2))  (rowwise scale)
  pooled = segment mean over seg_ids, then expmap0/proj (host epilogue)

Sharding: nodes split contiguously over cores (dst-shard). Each core owns
SHARD nodes, processes the edges whose dst is in its shard.  The spmm is a
one-hot matmul: for each 128-edge tile, S^T[e,slot] = (dstslot[e]==slot)
(DVE is_equal vs iota), stationary lhsT=S^T, moving rhs = gathered rows.
Gather via gpsimd.dma_gather with int16 indices (tables chunked to 32768
rows).  The only cross-core exchange is one AllGather of h1 (bf16).
"""

import math
from contextlib import ExitStack

import numpy as np
import ml_dtypes

import concourse.bass as bass
import concourse.tile as tile
import concourse.bacc as bacc
from concourse import mybir

BF16 = mybir.dt.bfloat16
F32 = mybir.dt.float32
I16 = mybir.dt.int16
AF = mybir.ActivationFunctionType
ALU = mybir.AluOpType

MAXNORM = 1.0 - 1e-5
MIN_SS = 1e-15

SUB = 3072          # gather indices per dma_gather call (descriptor ring limit)
GRP = 4             # dst blocks (of 128 nodes) per PSUM group
NREG = 116          # blocks per core with hard 512-edge/cell cap (rest overflow)


class Cfg:
    def __init__(self, n_nodes, in_dim, hid, n_seg, n_cores):
        self.N = n_nodes
        self.IN = in_dim
        self.HID = hid
        self.NSEG = n_seg
        self.NC = n_cores
        self.SHARD = n_nodes // n_cores
        assert self.SHARD % 128 == 0
        self.NBLK = self.SHARD // 128
        assert self.NBLK % GRP == 0
        self.NGRP = self.NBLK // GRP
        self.CH = min(32768, n_nodes)
        assert n_nodes % self.CH == 0
        self.NCHUNK = n_nodes // self.CH
        self.NSEGCH = (n_seg + 127) // 128


def _balance_core(d, nblk, cap=512, nreg=NREG):
    """Assign nodes (rows of d, [n,4] per-chunk in-degree) to nblk blocks of
    128 slots so that per-(block,chunk) edge counts stay <= cap for the first
    nreg blocks (overflow concentrated in the rest).  Returns pos[n] =
    block*128 + slot."""
    n = d.shape[0]
    tot = d.sum(1)
    order = np.argsort(-tot, kind="stable")
    cells = np.zeros((nblk, 4), np.int64)
    counts = np.zeros(nblk, np.int64)
    assign = np.empty(n, np.int64)
    for v in order:
        dv = d[v]
        c2 = cells + dv
        ok = (counts[:nreg] < 128) & (c2[:nreg] <= cap).all(1)
        if ok.any():
            cand = np.nonzero(ok)[0]
        else:
            cand = np.nonzero(counts < 128)[0]
        b = cand[np.argmin(c2[cand].max(1))]
        assign[v] = b
        counts[b] += 1
        cells[b] += dv
    pos = np.empty(n, np.int64)
    nxt = np.zeros(nblk, np.int64)
    for v in range(n):
        b = assign[v]
        pos[v] = b * 128 + nxt[b]
        nxt[b] += 1
    return pos


def host_prep(cfg, src, dst):
    """Build SPMD-uniform edge tiling + per-core index/slot arrays.

    Nodes are re-permuted within each core's shard (load balancing the
    (block, chunk) edge cells).  Returns (ntiles[NGRP,NCHUNK,GRP],
    per-core list of dicts with idx16_l1/idx16_l2 [128, TOT/16] int16 and
    dstslot [128, NTILES] float arrays, gpos[N] node->position map).
    """
    NC, SHARD, CH = cfg.NC, cfg.SHARD, cfg.CH
    src = np.asarray(src).astype(np.int64)
    dst = np.asarray(dst).astype(np.int64)

    chunk = src // CH

    # ---- balanced node -> (block, slot) permutation per core ----
    gpos = np.empty(cfg.N, np.int64)
    for c in range(NC):
        lo, hi = c * SHARD, (c + 1) * SHARD
        m = (dst >= lo) & (dst < hi)
        d_loc = np.zeros((SHARD, cfg.NCHUNK), np.int64)
        np.add.at(d_loc, (dst[m] - lo, chunk[m]), 1)
        gpos[lo:hi] = lo + _balance_core(d_loc, cfg.NBLK)

    pdst = gpos[dst]                    # permuted dst position
    core = pdst // SHARD
    blk = (pdst % SHARD) // 128         # block within core [0, NBLK)
    slot = pdst % 128
    idx = src % CH                      # layer-1 table index (x, id order)
    idx2 = gpos[src] % CH               # layer-2 table index (h1, pos order)

    # counts[c, g, k, b]
    counts = np.zeros((NC, cfg.NGRP, cfg.NCHUNK, GRP), dtype=np.int64)
    g_all = blk // GRP
    b_all = blk % GRP
    np.add.at(counts, (core, g_all, chunk, b_all), 1)

    mx = counts.max(axis=0)
    ntiles = (mx + 127) // 128
    # ensure every block has >= 1 tile in chunk 0 (so PSUM gets a start write)
    empty = ntiles.sum(axis=1) == 0      # [NGRP, GRP]
    ntiles[:, 0, :][empty] = 1

    NTILES = int(ntiles.sum())
    TOT = NTILES * 128

    per_core = []
    # canonical ordering: g, k, b, then edges of that cell (+pad)
    order = np.lexsort((idx, b_all, chunk, g_all, core))
    # cell boundaries per core
    for c in range(NC):
        idx16 = np.zeros(TOT, dtype=np.int16)
        idx16b = np.zeros(TOT, dtype=np.int16)
        slots = np.full(TOT, -1.0, dtype=np.float32)
        sel = order[core[order] == c]
        csrc_idx = idx[sel]
        csrc_idx2 = idx2[sel]
        cslot = slot[sel]
        cg = g_all[sel]
        ck = chunk[sel]
        cb = b_all[sel]
        # counts per cell for this core
        ccnt = counts[c]
        pos = 0      # position in canonical padded stream
        ep = 0       # position in sel
        for g in range(cfg.NGRP):
            for k in range(cfg.NCHUNK):
                for b in range(GRP):
                    n = int(ccnt[g, k, b])
                    cap = int(ntiles[g, k, b]) * 128
                    if n > 0:
                        idx16[pos:pos + n] = csrc_idx[ep:ep + n]
                        idx16b[pos:pos + n] = csrc_idx2[ep:ep + n]
                        slots[pos:pos + n] = cslot[ep:ep + n]
                        # sanity
                        assert np.all(cg[ep:ep + n] == g)
                        assert np.all(ck[ep:ep + n] == k)
                        assert np.all(cb[ep:ep + n] == b)
                        ep += n
                    pos += cap
        assert ep == len(sel)

        # wrap idx: i -> [i%16, i//16], replicate x8 partitions
        def wrap(a):
            iw = a.reshape(-1, 16).T            # [16, TOT/16]
            return np.tile(iw, (8, 1)).astype(np.int16)  # [128, TOT/16]

        # dstslot tile-major: [128 (edge in tile), NTILES]
        sl = slots.reshape(NTILES, 128).T.copy()
        per_core.append({"idx16_l1": wrap(idx16),
                         "idx16_l2": wrap(idx16b),
                         "dstslot": sl.astype(np.float32)})
    return ntiles, per_core, gpos


def _mm_schedule(cfg, ntiles):
    """Per (g): list over chunks of list of (tile_global_col, block b, start, stop)."""
    sched = []
    tcol = 0
    for g in range(cfg.NGRP):
        # first/last tile of each block across chunks
        tot_b = ntiles[g].sum(axis=0)   # [GRP]
        seen_b = np.zeros(GRP, dtype=np.int64)
        chunks = []
        for k in range(cfg.NCHUNK):
            tiles_k = []
            for b in range(GRP):
                for _ in range(int(ntiles[g, k, b])):
                    start = seen_b[b] == 0
                    stop = seen_b[b] == tot_b[b] - 1
                    tiles_k.append((tcol, b, bool(start), bool(stop)))
                    seen_b[b] += 1
                    tcol += 1
            chunks.append(tiles_k)
        sched.append(chunks)
    return sched


def build(cfg, ntiles, n_reps=1, debug_taps=False):
    """Build the Bass program. Returns nc."""
    N, IN, HID = cfg.N, cfg.IN, cfg.HID
    NTILES = int(ntiles.sum())
    TOT = NTILES * 128
    sched = _mm_schedule(cfg, ntiles)

    nc = bacc.Bacc("TRN2", target_bir_lowering=False,
                   dynamic_dma_scratch_size=65536)

    x_d = nc.dram_tensor("x_bf16", [N, IN], BF16, kind="ExternalInput")
    idx1_d = nc.dram_tensor("idx16_l1", [128, TOT // 16], I16, kind="ExternalInput")
    idx2_d = nc.dram_tensor("idx16_l2", [128, TOT // 16], I16, kind="ExternalInput")
    slot_d = nc.dram_tensor("dstslot", [128, NTILES], F32, kind="ExternalInput")
    segid_d = nc.dram_tensor("segid", [128, cfg.NBLK], F32, kind="ExternalInput")
    iota_d = nc.dram_tensor("iota128", [128, 128], BF16, kind="ExternalInput")
    iotas_d = nc.dram_tensor("iota_seg", [128, cfg.NSEGCH * 128], F32, kind="ExternalInput")
    ident_d = nc.dram_tensor("ident", [128, 128], BF16, kind="ExternalInput")
    w1_d = nc.dram_tensor("W1", [IN, HID], BF16, kind="ExternalInput")
    w2_d = nc.dram_tensor("W2", [HID, HID], BF16, kind="ExternalInput")
    b1_d = nc.dram_tensor("b1rep", [128, HID], F32, kind="ExternalInput")
    b2_d = nc.dram_tensor("b2rep", [128, HID], F32, kind="ExternalInput")

    h1_shard = nc.dram_tensor("h1_shard", [cfg.SHARD, HID], BF16)
    h1_full = nc.dram_tensor("h1_full", [N, HID], BF16, addr_space="Shared")
    out_d = nc.dram_tensor("pooled", [cfg.NSEGCH * 128, HID + 1], F32,
                           kind="ExternalOutput")
    if debug_taps:
        dbg_h1 = nc.dram_tensor("dbg_h1", [cfg.SHARD, HID], F32, kind="ExternalOutput")
        dbg_h2 = nc.dram_tensor("dbg_h2", [128, cfg.NBLK * HID], F32, kind="ExternalOutput")
        dbg_sc = nc.dram_tensor("dbg_sc", [128, 2 * cfg.NBLK], F32, kind="ExternalOutput")

    KIN = IN // 128   # k-chunks for W1 (2)

    with tile.TileContext(nc) as tc, ExitStack() as ctx:
        const = ctx.enter_context(tc.tile_pool(name="const", bufs=1))
        idxp = ctx.enter_context(tc.tile_pool(name="idxp", bufs=4))
        slotp = ctx.enter_context(tc.tile_pool(name="slotp", bufs=3))
        ebufp = ctx.enter_context(tc.tile_pool(name="ebufp", bufs=3))
        sp = ctx.enter_context(tc.tile_pool(name="sp", bufs=4))
        flshp = ctx.enter_context(tc.tile_pool(name="flshp", bufs=3))
        xtp = ctx.enter_context(tc.tile_pool(name="xtp", bufs=4))
        hp = ctx.enter_context(tc.tile_pool(name="hp", bufs=3))
        h2allp = ctx.enter_context(tc.tile_pool(name="h2allp", bufs=1))
        normp = ctx.enter_context(tc.tile_pool(name="normp", bufs=1))
        htp = ctx.enter_context(tc.tile_pool(name="htp", bufs=3))

        ctx_spmm = ctx.enter_context(ExitStack())
        ps_acc = ctx_spmm.enter_context(tc.tile_pool(name="ps_acc", bufs=4, space="PSUM"))
        ps_tr = ctx_spmm.enter_context(tc.tile_pool(name="ps_tr", bufs=1, space="PSUM"))
        ps_h = ctx_spmm.enter_context(tc.tile_pool(name="ps_h", bufs=2, space="PSUM"))

        # ---- constants ----
        iota128 = const.tile([128, 128], BF16)
        nc.sync.dma_start(iota128[:], iota_d[:])
        iotaseg = const.tile([128, cfg.NSEGCH * 128], F32)
        nc.sync.dma_start(iotaseg[:], iotas_d[:])
        ident = const.tile([128, 128], BF16)
        nc.sync.dma_start(ident[:], ident_d[:])
        segid = const.tile([128, cfg.NBLK], F32)
        nc.sync.dma_start(segid[:], segid_d[:])
        w1_sb = [const.tile([128, HID], BF16, tag=f"w1_{k}", name=f"w1_{k}")
                 for k in range(KIN)]
        for k in range(KIN):
            nc.sync.dma_start(w1_sb[k][:], w1_d[k * 128:(k + 1) * 128, :])
        w2_sb = const.tile([128, HID], BF16)
        nc.sync.dma_start(w2_sb[:], w2_d[:])
        b1_sb = const.tile([128, HID], F32)
        nc.sync.dma_start(b1_sb[:], b1_d[:])
        b2_sb = const.tile([128, HID], F32)
        nc.sync.dma_start(b2_sb[:], b2_d[:])

        h2_all = h2allp.tile([128, cfg.NBLK * HID], BF16)
        norms2 = normp.tile([128, cfg.NBLK], F32)
        scale = normp.tile([128, cfg.NBLK], F32)
        na = normp.tile([128, cfg.NBLK], F32)
        nb_t = normp.tile([128, cfg.NBLK], F32)

        def spmm_layer(layer, table_ap, idx_d, feat, out_block):
            """One spmm layer.  table_ap: DRAM [N, feat] gather table.
            out_block(g, b, agg_ps) consumes the accumulated [128(slot),
            feat] PSUM tile for global block nb=g*GRP+b.  One PSUM bank
            per block: start=True clears has_written bank-wide on HW, so
            accumulation groups must not share a bank."""
            for g in range(cfg.NGRP):
                accs = [ps_acc.tile([128, IN], F32, tag="acc", name=f"acc{b}")
                        for b in range(GRP)]

                def acc_slice(b):
                    return accs[b][:, :feat]

                for k in range(cfg.NCHUNK):
                    tiles_k = sched[g][k]
                    if not tiles_k:
                        continue
                    tbl = table_ap[k * cfg.CH:(k + 1) * cfg.CH, :]
                    # subcalls of <= SUB indices
                    for s0 in range(0, len(tiles_k), SUB // 128):
                        stiles = tiles_k[s0:s0 + SUB // 128]
                        nidx = len(stiles) * 128
                        col0 = stiles[0][0]  # global tile col
                        it = idxp.tile([128, SUB // 16], I16, tag="it")
                        nc.sync.dma_start(
                            it[:, :nidx // 16],
                            idx_d[:, col0 * 8:col0 * 8 + nidx // 16])
                        st = slotp.tile([128, SUB // 128], F32, tag="st")
                        nc.sync.dma_start(
                            st[:, :len(stiles)],
                            slot_d[:, col0:col0 + len(stiles)])
                        eb = ebufp.tile([128, (SUB // 128) * feat], BF16,
                                        tag=f"eb{layer}")
                        nc.gpsimd.dma_gather(
                            out_ap=eb[:, :len(stiles) * feat].rearrange(
                                "p (n f) -> p n f", f=feat),
                            in_ap=tbl,
                            idxs_ap=it[:, :nidx // 16],
                            num_idxs=nidx,
                            num_idxs_reg=nidx,
                            elem_size=feat,
                        )
                        for j, (tcol, b, st_f, sp_f) in enumerate(stiles):
                            s_t = sp.tile([128, 128], BF16, tag="s_t")
                            nc.vector.tensor_scalar(
                                s_t[:], iota128[:], st[:, j:j + 1], None,
                                ALU.is_equal)
                            nc.tensor.matmul(
                                acc_slice(b),
                                s_t[:],
                                eb[:, j * feat:(j + 1) * feat],
                                start=st_f, stop=sp_f,
                            )
                for b in range(GRP):
                    out_block(g, b, acc_slice(b))

        def l1_block(g, b, agg_ps):
            nb = g * GRP + b
            # copy PSUM f32 -> SBUF bf16
            ax = flshp.tile([128, IN], BF16, tag="ax1")
            nc.scalar.activation(ax[:], agg_ps, AF.Copy)
            h_ps = ps_h.tile([128, HID], F32, tag="hps", name="h_ps")
            for h in range(KIN):
                t_ps = ps_tr.tile([128, 128], BF16, tag="tps")
                nc.tensor.transpose(t_ps[:], ax[:, h * 128:(h + 1) * 128], ident[:])
                xt = xtp.tile([128, 128], BF16, tag="xt")
                nc.scalar.activation(xt[:], t_ps[:], AF.Copy)
                nc.tensor.matmul(h_ps[:], xt[:], w1_sb[h][:],
                                 start=(h == 0), stop=(h == KIN - 1))
            htmp = hp.tile([128, HID], F32, tag="htmp")
            nc.vector.tensor_add(htmp[:], h_ps[:], b1_sb[:])
            h1b = hp.tile([128, HID], BF16, tag="h1b")
            nc.scalar.activation(h1b[:], htmp[:], AF.Tanh)
            nc.sync.dma_start(h1_shard[nb * 128:(nb + 1) * 128, :], h1b[:])
            if debug_taps:
                h1f = hp.tile([128, HID], F32, tag="h1f")
                nc.scalar.activation(h1f[:], htmp[:], AF.Tanh)
                nc.sync.dma_start(dbg_h1[nb * 128:(nb + 1) * 128, :], h1f[:])

        def l2_block(g, b, agg_ps):
            nb = g * GRP + b
            a2 = flshp.tile([128, HID], BF16, tag="a22")
            nc.scalar.activation(a2[:], agg_ps, AF.Copy)
            t_ps = ps_tr.tile([128, 128], BF16, tag="tps")
            nc.tensor.transpose(t_ps[:], a2[:], ident[:])
            a2t = xtp.tile([128, 128], BF16, tag="xt")
            nc.scalar.activation(a2t[:], t_ps[:], AF.Copy)
            h_ps = ps_h.tile([128, HID], F32, tag="hps", name="h_ps")
            nc.tensor.matmul(h_ps[:], a2t[:], w2_sb[:], start=True, stop=True)
            htmp = hp.tile([128, HID], F32, tag="htmp")
            nc.vector.tensor_add(htmp[:], h_ps[:], b2_sb[:])
            nc.scalar.activation(h2_all[:, nb * HID:(nb + 1) * HID], htmp[:],
                                 AF.Tanh)

        # ---------------- layer 1 ----------------
        spmm_layer(1, x_d, idx1_d, IN, l1_block)

        # ---------------- exchange ----------------
        nc.gpsimd.collective_compute(
            "AllGather",
            ALU.bypass,
            ins=[h1_shard.ap().opt()],
            outs=[h1_full.ap().opt()],
            replica_groups=[list(range(cfg.NC))],
        )

        # ---------------- layer 2 ----------------
        spmm_layer(2, h1_full, idx2_d, HID, l2_block)

        # ---------------- norms + logmap scale ----------------
        for nbk in range(cfg.NBLK):
            h2b = h2_all[:, nbk * HID:(nbk + 1) * HID]
            sq = htp.tile([128, HID], F32, tag="sq")
            nc.vector.tensor_mul(sq[:], h2b, h2b)
            nc.vector.tensor_reduce(norms2[:, nbk:nbk + 1], sq[:],
                                    mybir.AxisListType.X, ALU.add)
        # norm = sqrt(max(ss, MIN_SS)); nclip = min(norm, MAXNORM)
        nc.vector.tensor_scalar_max(na[:], norms2[:], MIN_SS)
        nc.scalar.activation(nb_t[:], na[:], AF.Sqrt)        # nb_t = norm
        nc.vector.tensor_scalar_min(na[:], nb_t[:], MAXNORM)  # na = nclip
        # artanh(nclip) = 0.5*ln((1+n)/(1-n)); scale = artanh/norm
        one_m = normp.tile([128, cfg.NBLK], F32)
        nc.vector.tensor_scalar(one_m[:], na[:], -1.0, 1.0, ALU.mult, ALU.add)
        one_p = normp.tile([128, cfg.NBLK], F32)
        nc.vector.tensor_scalar_add(one_p[:], na[:], 1.0)
        rcp = normp.tile([128, cfg.NBLK], F32)
        nc.vector.reciprocal(rcp[:], one_m[:])
        rat = normp.tile([128, cfg.NBLK], F32)
        nc.vector.tensor_mul(rat[:], one_p[:], rcp[:])
        lg = normp.tile([128, cfg.NBLK], F32)
        nc.scalar.activation(lg[:], rat[:], AF.Ln)
        nc.vector.tensor_scalar_mul(lg[:], lg[:], 0.5)
        rcpn = normp.tile([128, cfg.NBLK], F32)
        nc.vector.reciprocal(rcpn[:], nb_t[:])
        nc.vector.tensor_mul(scale[:], lg[:], rcpn[:])

        if debug_taps:
            nc.sync.dma_start(dbg_h2[:], h2_all[:])
            nc.sync.dma_start(dbg_sc[:, :cfg.NBLK], norms2[:])
            nc.sync.dma_start(dbg_sc[:, cfg.NBLK:], scale[:])
        # ---------------- pooling ----------------
        ctx_spmm.close()
        ps_pool = ctx.enter_context(
            tc.tile_pool(name="ps_pool", bufs=max(cfg.NSEGCH, 1), space="PSUM"))
        pool_ps = [ps_pool.tile([128, HID + 1], F32, tag="pool", name=f"pool{sc}")
                   for sc in range(cfg.NSEGCH)]
        for nbk in range(cfg.NBLK):
            h2b = h2_all[:, nbk * HID:(nbk + 1) * HID]
            ht = htp.tile([128, HID + 1], BF16, tag="ht")
            nc.vector.tensor_scalar(ht[:, :HID], h2b, scale[:, nbk:nbk + 1],
                                    None, ALU.mult)
            nc.vector.memset(ht[:, HID:HID + 1], 1.0)
            for sc in range(cfg.NSEGCH):
                sg = sp.tile([128, 128], BF16, tag="sg")
                nc.vector.tensor_scalar(
                    sg[:], iotaseg[:, sc * 128:(sc + 1) * 128],
                    segid[:, nbk:nbk + 1], None, ALU.is_equal)
                nc.tensor.matmul(
                    pool_ps[sc][:], sg[:], ht[:],
                    start=(nbk == 0), stop=(nbk == cfg.NBLK - 1))
        for sc in range(cfg.NSEGCH):
            po = htp.tile([128, HID + 1], F32, tag="po")
            nc.vector.tensor_copy(po[:], pool_ps[sc][:])
            nc.sync.dma_start(out_d[sc * 128:(sc + 1) * 128, :], po[:])

    nc.compile()
    return nc


def host_inputs(cfg, x, seg_ids, W1, b1, W2, b2, per_core, gpos):
    """Per-core in_maps for run_bass_kernel_spmd."""
    N, IN, HID = cfg.N, cfg.IN, cfg.HID
    x_bf16 = np.ascontiguousarray(x.astype(ml_dtypes.bfloat16))
    iota128 = np.tile(np.arange(128, dtype=np.float32), (128, 1)).astype(ml_dtypes.bfloat16)
    iotaseg = np.tile(np.arange(cfg.NSEGCH * 128, dtype=np.float32), (128, 1))
    ident = np.eye(128, dtype=np.float32).astype(ml_dtypes.bfloat16)
    w1 = np.ascontiguousarray(W1.astype(ml_dtypes.bfloat16))
    w2 = np.ascontiguousarray(W2.astype(ml_dtypes.bfloat16))
    b1r = np.tile(np.asarray(b1, np.float32), (128, 1))
    b2r = np.tile(np.asarray(b2, np.float32), (128, 1))
    seg_pos = np.empty(cfg.N, np.float32)
    seg_pos[gpos] = np.asarray(seg_ids, np.float32)   # seg id by position
    maps = []
    for c in range(cfg.NC):
        segc = seg_pos[c * cfg.SHARD:(c + 1) * cfg.SHARD].reshape(cfg.NBLK, 128).T
        maps.append({
            "x_bf16": x_bf16,
            "idx16_l1": per_core[c]["idx16_l1"],
            "idx16_l2": per_core[c]["idx16_l2"],
            "dstslot": per_core[c]["dstslot"],
            "segid": np.ascontiguousarray(segc),
            "iota128": iota128,
            "iota_seg": np.ascontiguousarray(iotaseg.astype(np.float32)),
            "ident": ident,
            "W1": w1,
            "W2": w2,
            "b1rep": b1r,
            "b2rep": b2r,
        })
    return maps


def host_epilogue(cfg, partials, batch_size, max_comments):
    """partials: list of per-core [NSEGCH*128, HID+1] f32."""
    acc = np.zeros_like(partials[0], dtype=np.float64)
    for p in partials:
        acc += p.astype(np.float64)
    acc = acc.astype(np.float32)
    nseg = cfg.NSEG
    sums = acc[:nseg, :cfg.HID]
    counts = acc[:nseg, cfg.HID]
    agg = sums / np.maximum(counts, 1.0)[:, None]
    # expmap0 then proj
    ss = np.maximum(np.sum(agg * agg, axis=1), MIN_SS).astype(np.float32)
    norm = np.sqrt(ss)
    y = agg * (np.tanh(norm) / norm)[:, None]
    ssy = np.maximum(np.sum(y * y, axis=1), MIN_SS).astype(np.float32)
    ny = np.sqrt(ssy)
    f = np.where(ny > MAXNORM, MAXNORM / ny, 1.0).astype(np.float32)
    y = y * f[:, None]
    return y.reshape(int(batch_size), int(max_comments), cfg.HID)


# ---------------- numpy reference (for arbitrary sizes) ----------------

def np_reference(x, src, dst, seg_ids, W1, b1, W2, b2, batch_size, max_comments):
    n = x.shape[0]

    def seg_sum(vals, ids, nseg):
        out = np.zeros((nseg, vals.shape[1]), np.float32)
        np.add.at(out, ids, vals)
        return out

    def rownorm(v):
        return np.sqrt(np.maximum(np.sum(v * v, axis=1, keepdims=True), MIN_SS))

    def proj(v):
        nn = rownorm(v)
        return np.where(nn > MAXNORM, v / nn * MAXNORM, v)

    def logmap0(v):
        nn = rownorm(v)
        arg = np.minimum(nn, 1 - 1e-7)
        return v * np.arctanh(arg) / nn

    def expmap0(v):
        nn = rownorm(v)
        return v * np.tanh(nn) / nn

    h = np.tanh(seg_sum(x[src] @ W1, dst, n) + b1)
    h = np.tanh(seg_sum(h[src] @ W2, dst, n) + b2)
    h = logmap0(proj(h))
    nseg = int(batch_size) * int(max_comments)
    sums = seg_sum(h, seg_ids, nseg)
    counts = np.zeros(nseg, np.float32)
    np.add.at(counts, seg_ids, 1.0)
    agg = sums / np.maximum(counts, 1.0)[:, None]
    agg = proj(expmap0(agg))
    return agg.reshape(int(batch_size), int(max_comments), -1)


# ====================================================================
# Harness entry point: kernel(**inputs) -> np.ndarray
# ====================================================================

_CACHE = {}


def kernel(x, src, dst, seg_ids, W1, b1, W2, b2, batch_size, max_comments):
    """Full-input GNN ComEnc kernel on 8 Trainium2 NeuronCores.

    Accepts the unsharded inputs of reference.setup_inputs() and returns
    the full (batch, max_comments, HID) float32 output.
    """
    from concourse.bass_utils import run_bass_kernel_spmd

    x = np.asarray(x, dtype=np.float32)
    src = np.asarray(src).astype(np.int64)
    dst = np.asarray(dst).astype(np.int64)
    seg_ids = np.asarray(seg_ids).astype(np.int64)
    W1 = np.asarray(W1, dtype=np.float32)
    b1 = np.asarray(b1, dtype=np.float32)
    W2 = np.asarray(W2, dtype=np.float32)
    b2 = np.asarray(b2, dtype=np.float32)
    bs = int(np.asarray(batch_size))
    mc = int(np.asarray(max_comments))

    n_nodes, in_dim = x.shape
    hid = W1.shape[1]
    nseg = bs * mc
    n_cores = 8

    cfg = Cfg(n_nodes, in_dim, hid, nseg, n_cores)
    ntiles, per_core, gpos = host_prep(cfg, src, dst)

    key = (n_nodes, in_dim, hid, nseg, ntiles.tobytes())
    if key in _CACHE:
        nc = _CACHE[key]
    else:
        nc = build(cfg, ntiles)
        _CACHE.clear()
        _CACHE[key] = nc

    maps = host_inputs(cfg, x, seg_ids, W1, b1, W2, b2, per_core, gpos)
    res = run_bass_kernel_spmd(nc, maps, core_ids=list(range(n_cores)))
    partials = [r["pooled"] for r in res.results]
    out = host_epilogue(cfg, partials, bs, mc)
    return np.ascontiguousarray(out.astype(np.float32))



# revision 14
# speedup vs baseline: 1.4113x; 1.0652x over previous
"""GNN message-passing kernel for Trainium2 (8 NeuronCores, SPMD).

Computation (see reference):
  h1 = tanh(segsum(x[src] -> dst) @ W1 + b1)        [uses A(xW) = (Ax)W]
  h2 = tanh(segsum(h1[src] -> dst) @ W2 + b2)
  ht = logmap0(proj(h2))  (rowwise scale)
  pooled = segment mean over seg_ids, then expmap0/proj (host epilogue)

Sharding: nodes split contiguously over cores (dst-shard). Each core owns
SHARD nodes, processes the edges whose dst is in its shard.  The spmm is a
one-hot matmul: for each 128-edge tile, S^T[e,slot] = (dstslot[e]==slot)
(DVE is_equal vs iota), stationary lhsT=S^T, moving rhs = gathered rows.
Gather via gpsimd.dma_gather with int16 indices (tables chunked to 32768
rows).  The only cross-core exchange is one AllGather of h1 (bf16).
"""

import math
from contextlib import ExitStack

import numpy as np
import ml_dtypes

import concourse.bass as bass
import concourse.tile as tile
import concourse.bacc as bacc
from concourse import mybir

BF16 = mybir.dt.bfloat16
F32 = mybir.dt.float32
I16 = mybir.dt.int16
AF = mybir.ActivationFunctionType
ALU = mybir.AluOpType

MAXNORM = 1.0 - 1e-5
MIN_SS = 1e-15

SUB = 3072          # gather indices per dma_gather call (descriptor ring limit)
GRP = 4             # dst blocks (of 128 nodes) per PSUM group
NREG = 126         # blocks per core with hard 512-edge/cell cap (rest overflow)


class Cfg:
    def __init__(self, n_nodes, in_dim, hid, n_seg, n_cores):
        self.N = n_nodes
        self.IN = in_dim
        self.HID = hid
        self.NSEG = n_seg
        self.NC = n_cores
        self.SHARD = n_nodes // n_cores
        assert self.SHARD % 128 == 0
        self.NBLK = self.SHARD // 128
        assert self.NBLK % GRP == 0
        self.NGRP = self.NBLK // GRP
        self.CH = min(32768, n_nodes)
        assert n_nodes % self.CH == 0
        self.NCHUNK = n_nodes // self.CH
        self.NSEGCH = (n_seg + 127) // 128


def _balance_core(d, nblk, cap=512, nreg=NREG):
    """Assign nodes (rows of d, [n,4] per-chunk in-degree) to nblk blocks of
    128 slots so that per-(block,chunk) edge counts stay <= cap for the first
    nreg blocks (overflow concentrated in the rest).  Returns pos[n] =
    block*128 + slot."""
    n = d.shape[0]
    tot = d.sum(1)
    order = np.argsort(-tot, kind="stable")
    novf = (nblk - nreg) * 128          # highest-degree nodes -> overflow
    cells = np.zeros((nblk, 4), np.int64)
    counts = np.zeros(nblk, np.int64)
    assign = np.empty(n, np.int64)
    for i in range(n):
        v = order[i]
        dv = d[v]
        c2 = cells + dv
        if i < novf:
            cand = np.nonzero(counts[nreg:] < 128)[0] + nreg
        else:
            ok = (counts[:nreg] < 128) & (c2[:nreg] <= cap).all(1)
            cand = np.nonzero(ok)[0]
            if not len(cand):
                cand = np.nonzero(counts < 128)[0]
        b = cand[np.argmin(c2[cand].max(1))]
        assign[v] = b
        counts[b] += 1
        cells[b] += dv
    pos = np.empty(n, np.int64)
    nxt = np.zeros(nblk, np.int64)
    for v in range(n):
        b = assign[v]
        pos[v] = b * 128 + nxt[b]
        nxt[b] += 1
    return pos


def host_prep(cfg, src, dst):
    """Build SPMD-uniform edge tiling + per-core index/slot arrays.

    Nodes are re-permuted within each core's shard (load balancing the
    (block, chunk) edge cells).  Returns (ntiles[NGRP,NCHUNK,GRP],
    per-core list of dicts with idx16_l1/idx16_l2 [128, TOT/16] int16 and
    dstslot [128, NTILES] float arrays, gpos[N] node->position map).
    """
    NC, SHARD, CH = cfg.NC, cfg.SHARD, cfg.CH
    src = np.asarray(src).astype(np.int64)
    dst = np.asarray(dst).astype(np.int64)

    chunk = src // CH

    # ---- balanced node -> (block, slot) permutation per core ----
    gpos = np.empty(cfg.N, np.int64)
    for c in range(NC):
        lo, hi = c * SHARD, (c + 1) * SHARD
        m = (dst >= lo) & (dst < hi)
        d_loc = np.zeros((SHARD, cfg.NCHUNK), np.int64)
        np.add.at(d_loc, (dst[m] - lo, chunk[m]), 1)
        gpos[lo:hi] = lo + _balance_core(d_loc, cfg.NBLK)

    pdst = gpos[dst]                    # permuted dst position
    core = pdst // SHARD
    blk = (pdst % SHARD) // 128         # block within core [0, NBLK)
    slot = pdst % 128
    idx = src % CH                      # layer-1 table index (x, id order)
    idx2 = gpos[src] % CH               # layer-2 table index (h1, pos order)

    # counts[c, g, k, b]
    counts = np.zeros((NC, cfg.NGRP, cfg.NCHUNK, GRP), dtype=np.int64)
    g_all = blk // GRP
    b_all = blk % GRP
    np.add.at(counts, (core, g_all, chunk, b_all), 1)

    mx = counts.max(axis=0)
    ntiles = (mx + 127) // 128
    # ensure every block has >= 1 tile in chunk 0 (so PSUM gets a start write)
    empty = ntiles.sum(axis=1) == 0      # [NGRP, GRP]
    ntiles[:, 0, :][empty] = 1

    NTILES = int(ntiles.sum())
    TOT = NTILES * 128

    per_core = []
    # canonical ordering: g, k, b, then edges of that cell (+pad)
    order = np.lexsort((idx, b_all, chunk, g_all, core))
    # cell boundaries per core
    for c in range(NC):
        idx16 = np.zeros(TOT, dtype=np.int16)
        idx16b = np.zeros(TOT, dtype=np.int16)
        slots = np.full(TOT, -1.0, dtype=np.float32)
        sel = order[core[order] == c]
        csrc_idx = idx[sel]
        csrc_idx2 = idx2[sel]
        cslot = slot[sel]
        cg = g_all[sel]
        ck = chunk[sel]
        cb = b_all[sel]
        # counts per cell for this core
        ccnt = counts[c]
        pos = 0      # position in canonical padded stream
        ep = 0       # position in sel
        for g in range(cfg.NGRP):
            for k in range(cfg.NCHUNK):
                for b in range(GRP):
                    n = int(ccnt[g, k, b])
                    cap = int(ntiles[g, k, b]) * 128
                    if n > 0:
                        idx16[pos:pos + n] = csrc_idx[ep:ep + n]
                        idx16b[pos:pos + n] = csrc_idx2[ep:ep + n]
                        slots[pos:pos + n] = cslot[ep:ep + n]
                        # sanity
                        assert np.all(cg[ep:ep + n] == g)
                        assert np.all(ck[ep:ep + n] == k)
                        assert np.all(cb[ep:ep + n] == b)
                        ep += n
                    pos += cap
        assert ep == len(sel)

        # wrap idx: i -> [i%16, i//16], replicate x8 partitions
        def wrap(a):
            iw = a.reshape(-1, 16).T            # [16, TOT/16]
            return np.tile(iw, (8, 1)).astype(np.int16)  # [128, TOT/16]

        # dstslot tile-major: [128 (edge in tile), NTILES]
        sl = slots.reshape(NTILES, 128).T.copy()
        per_core.append({"idx16_l1": wrap(idx16),
                         "idx16_l2": wrap(idx16b),
                         "dstslot": sl.astype(np.float32)})
    return ntiles, per_core, gpos


def _mm_schedule(cfg, ntiles):
    """Per (g): list over chunks of list of (tile_global_col, block b, start, stop)."""
    sched = []
    tcol = 0
    for g in range(cfg.NGRP):
        # first/last tile of each block across chunks
        tot_b = ntiles[g].sum(axis=0)   # [GRP]
        seen_b = np.zeros(GRP, dtype=np.int64)
        chunks = []
        for k in range(cfg.NCHUNK):
            tiles_k = []
            for b in range(GRP):
                for _ in range(int(ntiles[g, k, b])):
                    start = seen_b[b] == 0
                    stop = seen_b[b] == tot_b[b] - 1
                    tiles_k.append((tcol, b, bool(start), bool(stop)))
                    seen_b[b] += 1
                    tcol += 1
            chunks.append(tiles_k)
        sched.append(chunks)
    return sched


def build(cfg, ntiles, n_reps=1, debug_taps=False):
    """Build the Bass program. Returns nc."""
    N, IN, HID = cfg.N, cfg.IN, cfg.HID
    NTILES = int(ntiles.sum())
    TOT = NTILES * 128
    sched = _mm_schedule(cfg, ntiles)

    nc = bacc.Bacc("TRN2", target_bir_lowering=False,
                   dynamic_dma_scratch_size=65536)

    x_d = nc.dram_tensor("x_bf16", [N, IN], BF16, kind="ExternalInput")
    idx1_d = nc.dram_tensor("idx16_l1", [128, TOT // 16], I16, kind="ExternalInput")
    idx2_d = nc.dram_tensor("idx16_l2", [128, TOT // 16], I16, kind="ExternalInput")
    slot_d = nc.dram_tensor("dstslot", [128, NTILES], F32, kind="ExternalInput")
    segid_d = nc.dram_tensor("segid", [128, cfg.NBLK], F32, kind="ExternalInput")
    iota_d = nc.dram_tensor("iota128", [128, 128], BF16, kind="ExternalInput")
    iotas_d = nc.dram_tensor("iota_seg", [128, cfg.NSEGCH * 128], F32, kind="ExternalInput")
    ident_d = nc.dram_tensor("ident", [128, 128], BF16, kind="ExternalInput")
    w1_d = nc.dram_tensor("W1", [IN, HID], BF16, kind="ExternalInput")
    w2_d = nc.dram_tensor("W2", [HID, HID], BF16, kind="ExternalInput")
    b1_d = nc.dram_tensor("b1rep", [128, HID], F32, kind="ExternalInput")
    b2_d = nc.dram_tensor("b2rep", [128, HID], F32, kind="ExternalInput")

    h1_shard = nc.dram_tensor("h1_shard", [cfg.SHARD, HID], BF16)
    h1_full = nc.dram_tensor("h1_full", [N, HID], BF16, addr_space="Shared")
    out_d = nc.dram_tensor("pooled", [cfg.NSEGCH * 128, HID + 1], F32,
                           kind="ExternalOutput")
    if debug_taps:
        dbg_h1 = nc.dram_tensor("dbg_h1", [cfg.SHARD, HID], F32, kind="ExternalOutput")
        dbg_h2 = nc.dram_tensor("dbg_h2", [128, cfg.NBLK * HID], F32, kind="ExternalOutput")
        dbg_sc = nc.dram_tensor("dbg_sc", [128, 2 * cfg.NBLK], F32, kind="ExternalOutput")

    KIN = IN // 128   # k-chunks for W1 (2)

    with tile.TileContext(nc) as tc, ExitStack() as ctx:
        const = ctx.enter_context(tc.tile_pool(name="const", bufs=1))
        idxp = ctx.enter_context(tc.tile_pool(name="idxp", bufs=4))
        slotp = ctx.enter_context(tc.tile_pool(name="slotp", bufs=3))
        ebufp = ctx.enter_context(tc.tile_pool(name="ebufp", bufs=3))
        sp = ctx.enter_context(tc.tile_pool(name="sp", bufs=4))
        flshp = ctx.enter_context(tc.tile_pool(name="flshp", bufs=3))
        xtp = ctx.enter_context(tc.tile_pool(name="xtp", bufs=4))
        hp = ctx.enter_context(tc.tile_pool(name="hp", bufs=3))
        h2allp = ctx.enter_context(tc.tile_pool(name="h2allp", bufs=1))
        normp = ctx.enter_context(tc.tile_pool(name="normp", bufs=1))
        htp = ctx.enter_context(tc.tile_pool(name="htp", bufs=3))

        ctx_spmm = ctx.enter_context(ExitStack())
        ps_acc = ctx_spmm.enter_context(tc.tile_pool(name="ps_acc", bufs=4, space="PSUM"))
        ps_tr = ctx_spmm.enter_context(tc.tile_pool(name="ps_tr", bufs=1, space="PSUM"))
        ps_h = ctx_spmm.enter_context(tc.tile_pool(name="ps_h", bufs=2, space="PSUM"))

        # ---- constants ----
        iota128 = const.tile([128, 128], BF16)
        nc.sync.dma_start(iota128[:], iota_d[:])
        iotaseg = const.tile([128, cfg.NSEGCH * 128], F32)
        nc.sync.dma_start(iotaseg[:], iotas_d[:])
        ident = const.tile([128, 128], BF16)
        nc.sync.dma_start(ident[:], ident_d[:])
        segid = const.tile([128, cfg.NBLK], F32)
        nc.sync.dma_start(segid[:], segid_d[:])
        w1_sb = [const.tile([128, HID], BF16, tag=f"w1_{k}", name=f"w1_{k}")
                 for k in range(KIN)]
        for k in range(KIN):
            nc.sync.dma_start(w1_sb[k][:], w1_d[k * 128:(k + 1) * 128, :])
        w2_sb = const.tile([128, HID], BF16)
        nc.sync.dma_start(w2_sb[:], w2_d[:])
        b1_sb = const.tile([128, HID], F32)
        nc.sync.dma_start(b1_sb[:], b1_d[:])
        b2_sb = const.tile([128, HID], F32)
        nc.sync.dma_start(b2_sb[:], b2_d[:])

        h2_all = h2allp.tile([128, cfg.NBLK * HID], BF16)
        norms2 = normp.tile([128, cfg.NBLK], F32)
        scale = normp.tile([128, cfg.NBLK], F32)
        na = normp.tile([128, cfg.NBLK], F32)
        nb_t = normp.tile([128, cfg.NBLK], F32)

        def spmm_layer(layer, table_ap, idx_d, feat, out_block):
            """One spmm layer.  table_ap: DRAM [N, feat] gather table.
            out_block(g, b, agg_ps) consumes the accumulated [128(slot),
            feat] PSUM tile for global block nb=g*GRP+b.  One PSUM bank
            per block: start=True clears has_written bank-wide on HW, so
            accumulation groups must not share a bank."""
            for g in range(cfg.NGRP):
                accs = [ps_acc.tile([128, IN], F32, tag="acc", name=f"acc{b}")
                        for b in range(GRP)]

                def acc_slice(b):
                    return accs[b][:, :feat]

                for k in range(cfg.NCHUNK):
                    tiles_k = sched[g][k]
                    if not tiles_k:
                        continue
                    tbl = table_ap[k * cfg.CH:(k + 1) * cfg.CH, :]
                    # subcalls of <= SUB indices
                    for s0 in range(0, len(tiles_k), SUB // 128):
                        stiles = tiles_k[s0:s0 + SUB // 128]
                        nidx = len(stiles) * 128
                        col0 = stiles[0][0]  # global tile col
                        it = idxp.tile([128, SUB // 16], I16, tag="it")
                        nc.sync.dma_start(
                            it[:, :nidx // 16],
                            idx_d[:, col0 * 8:col0 * 8 + nidx // 16])
                        st = slotp.tile([128, SUB // 128], F32, tag="st")
                        nc.sync.dma_start(
                            st[:, :len(stiles)],
                            slot_d[:, col0:col0 + len(stiles)])
                        eb = ebufp.tile([128, (SUB // 128) * feat], BF16,
                                        tag=f"eb{layer}")
                        nc.gpsimd.dma_gather(
                            out_ap=eb[:, :len(stiles) * feat].rearrange(
                                "p (n f) -> p n f", f=feat),
                            in_ap=tbl,
                            idxs_ap=it[:, :nidx // 16],
                            num_idxs=nidx,
                            num_idxs_reg=nidx,
                            elem_size=feat,
                        )
                        for j, (tcol, b, st_f, sp_f) in enumerate(stiles):
                            s_t = sp.tile([128, 128], BF16, tag="s_t")
                            nc.vector.tensor_scalar(
                                s_t[:], iota128[:], st[:, j:j + 1], None,
                                ALU.is_equal)
                            nc.tensor.matmul(
                                acc_slice(b),
                                s_t[:],
                                eb[:, j * feat:(j + 1) * feat],
                                start=st_f, stop=sp_f,
                            )
                for b in range(GRP):
                    out_block(g, b, acc_slice(b))

        def l1_block(g, b, agg_ps):
            nb = g * GRP + b
            # copy PSUM f32 -> SBUF bf16
            ax = flshp.tile([128, IN], BF16, tag="ax1")
            nc.scalar.activation(ax[:], agg_ps, AF.Copy)
            h_ps = ps_h.tile([128, HID], F32, tag="hps", name="h_ps")
            for h in range(KIN):
                t_ps = ps_tr.tile([128, 128], BF16, tag="tps")
                nc.tensor.transpose(t_ps[:], ax[:, h * 128:(h + 1) * 128], ident[:])
                xt = xtp.tile([128, 128], BF16, tag="xt")
                nc.scalar.activation(xt[:], t_ps[:], AF.Copy)
                nc.tensor.matmul(h_ps[:], xt[:], w1_sb[h][:],
                                 start=(h == 0), stop=(h == KIN - 1))
            htmp = hp.tile([128, HID], F32, tag="htmp")
            nc.vector.tensor_add(htmp[:], h_ps[:], b1_sb[:])
            h1b = hp.tile([128, HID], BF16, tag="h1b")
            nc.scalar.activation(h1b[:], htmp[:], AF.Tanh)
            nc.sync.dma_start(h1_shard[nb * 128:(nb + 1) * 128, :], h1b[:])
            if debug_taps:
                h1f = hp.tile([128, HID], F32, tag="h1f")
                nc.scalar.activation(h1f[:], htmp[:], AF.Tanh)
                nc.sync.dma_start(dbg_h1[nb * 128:(nb + 1) * 128, :], h1f[:])

        def l2_block(g, b, agg_ps):
            nb = g * GRP + b
            a2 = flshp.tile([128, HID], BF16, tag="a22")
            nc.scalar.activation(a2[:], agg_ps, AF.Copy)
            t_ps = ps_tr.tile([128, 128], BF16, tag="tps")
            nc.tensor.transpose(t_ps[:], a2[:], ident[:])
            a2t = xtp.tile([128, 128], BF16, tag="xt")
            nc.scalar.activation(a2t[:], t_ps[:], AF.Copy)
            h_ps = ps_h.tile([128, HID], F32, tag="hps", name="h_ps")
            nc.tensor.matmul(h_ps[:], a2t[:], w2_sb[:], start=True, stop=True)
            htmp = hp.tile([128, HID], F32, tag="htmp")
            nc.vector.tensor_add(htmp[:], h_ps[:], b2_sb[:])
            nc.scalar.activation(h2_all[:, nb * HID:(nb + 1) * HID], htmp[:],
                                 AF.Tanh)

        # ---------------- layer 1 ----------------
        spmm_layer(1, x_d, idx1_d, IN, l1_block)

        # ---------------- exchange ----------------
        nc.gpsimd.collective_compute(
            "AllGather",
            ALU.bypass,
            ins=[h1_shard.ap().opt()],
            outs=[h1_full.ap().opt()],
            replica_groups=[list(range(cfg.NC))],
        )

        # ---------------- layer 2 ----------------
        spmm_layer(2, h1_full, idx2_d, HID, l2_block)

        # ---------------- norms + logmap scale ----------------
        for nbk in range(cfg.NBLK):
            h2b = h2_all[:, nbk * HID:(nbk + 1) * HID]
            sq = htp.tile([128, HID], F32, tag="sq")
            nc.vector.tensor_mul(sq[:], h2b, h2b)
            nc.vector.tensor_reduce(norms2[:, nbk:nbk + 1], sq[:],
                                    mybir.AxisListType.X, ALU.add)
        # norm = sqrt(max(ss, MIN_SS)); nclip = min(norm, MAXNORM)
        nc.vector.tensor_scalar_max(na[:], norms2[:], MIN_SS)
        nc.scalar.activation(nb_t[:], na[:], AF.Sqrt)        # nb_t = norm
        nc.vector.tensor_scalar_min(na[:], nb_t[:], MAXNORM)  # na = nclip
        # artanh(nclip) = 0.5*ln((1+n)/(1-n)); scale = artanh/norm
        one_m = normp.tile([128, cfg.NBLK], F32)
        nc.vector.tensor_scalar(one_m[:], na[:], -1.0, 1.0, ALU.mult, ALU.add)
        one_p = normp.tile([128, cfg.NBLK], F32)
        nc.vector.tensor_scalar_add(one_p[:], na[:], 1.0)
        rcp = normp.tile([128, cfg.NBLK], F32)
        nc.vector.reciprocal(rcp[:], one_m[:])
        rat = normp.tile([128, cfg.NBLK], F32)
        nc.vector.tensor_mul(rat[:], one_p[:], rcp[:])
        lg = normp.tile([128, cfg.NBLK], F32)
        nc.scalar.activation(lg[:], rat[:], AF.Ln)
        nc.vector.tensor_scalar_mul(lg[:], lg[:], 0.5)
        rcpn = normp.tile([128, cfg.NBLK], F32)
        nc.vector.reciprocal(rcpn[:], nb_t[:])
        nc.vector.tensor_mul(scale[:], lg[:], rcpn[:])

        if debug_taps:
            nc.sync.dma_start(dbg_h2[:], h2_all[:])
            nc.sync.dma_start(dbg_sc[:, :cfg.NBLK], norms2[:])
            nc.sync.dma_start(dbg_sc[:, cfg.NBLK:], scale[:])
        # ---------------- pooling ----------------
        ctx_spmm.close()
        ps_pool = ctx.enter_context(
            tc.tile_pool(name="ps_pool", bufs=max(cfg.NSEGCH, 1), space="PSUM"))
        pool_ps = [ps_pool.tile([128, HID + 1], F32, tag="pool", name=f"pool{sc}")
                   for sc in range(cfg.NSEGCH)]
        for nbk in range(cfg.NBLK):
            h2b = h2_all[:, nbk * HID:(nbk + 1) * HID]
            ht = htp.tile([128, HID + 1], BF16, tag="ht")
            nc.vector.tensor_scalar(ht[:, :HID], h2b, scale[:, nbk:nbk + 1],
                                    None, ALU.mult)
            nc.vector.memset(ht[:, HID:HID + 1], 1.0)
            for sc in range(cfg.NSEGCH):
                sg = sp.tile([128, 128], BF16, tag="sg")
                nc.vector.tensor_scalar(
                    sg[:], iotaseg[:, sc * 128:(sc + 1) * 128],
                    segid[:, nbk:nbk + 1], None, ALU.is_equal)
                nc.tensor.matmul(
                    pool_ps[sc][:], sg[:], ht[:],
                    start=(nbk == 0), stop=(nbk == cfg.NBLK - 1))
        for sc in range(cfg.NSEGCH):
            po = htp.tile([128, HID + 1], F32, tag="po")
            nc.vector.tensor_copy(po[:], pool_ps[sc][:])
            nc.sync.dma_start(out_d[sc * 128:(sc + 1) * 128, :], po[:])

    nc.compile()
    return nc


def host_inputs(cfg, x, seg_ids, W1, b1, W2, b2, per_core, gpos):
    """Per-core in_maps for run_bass_kernel_spmd."""
    N, IN, HID = cfg.N, cfg.IN, cfg.HID
    x_bf16 = np.ascontiguousarray(x.astype(ml_dtypes.bfloat16))
    iota128 = np.tile(np.arange(128, dtype=np.float32), (128, 1)).astype(ml_dtypes.bfloat16)
    iotaseg = np.tile(np.arange(cfg.NSEGCH * 128, dtype=np.float32), (128, 1))
    ident = np.eye(128, dtype=np.float32).astype(ml_dtypes.bfloat16)
    w1 = np.ascontiguousarray(W1.astype(ml_dtypes.bfloat16))
    w2 = np.ascontiguousarray(W2.astype(ml_dtypes.bfloat16))
    b1r = np.tile(np.asarray(b1, np.float32), (128, 1))
    b2r = np.tile(np.asarray(b2, np.float32), (128, 1))
    seg_pos = np.empty(cfg.N, np.float32)
    seg_pos[gpos] = np.asarray(seg_ids, np.float32)   # seg id by position
    maps = []
    for c in range(cfg.NC):
        segc = seg_pos[c * cfg.SHARD:(c + 1) * cfg.SHARD].reshape(cfg.NBLK, 128).T
        maps.append({
            "x_bf16": x_bf16,
            "idx16_l1": per_core[c]["idx16_l1"],
            "idx16_l2": per_core[c]["idx16_l2"],
            "dstslot": per_core[c]["dstslot"],
            "segid": np.ascontiguousarray(segc),
            "iota128": iota128,
            "iota_seg": np.ascontiguousarray(iotaseg.astype(np.float32)),
            "ident": ident,
            "W1": w1,
            "W2": w2,
            "b1rep": b1r,
            "b2rep": b2r,
        })
    return maps


def host_epilogue(cfg, partials, batch_size, max_comments):
    """partials: list of per-core [NSEGCH*128, HID+1] f32."""
    acc = np.zeros_like(partials[0], dtype=np.float64)
    for p in partials:
        acc += p.astype(np.float64)
    acc = acc.astype(np.float32)
    nseg = cfg.NSEG
    sums = acc[:nseg, :cfg.HID]
    counts = acc[:nseg, cfg.HID]
    agg = sums / np.maximum(counts, 1.0)[:, None]
    # expmap0 then proj
    ss = np.maximum(np.sum(agg * agg, axis=1), MIN_SS).astype(np.float32)
    norm = np.sqrt(ss)
    y = agg * (np.tanh(norm) / norm)[:, None]
    ssy = np.maximum(np.sum(y * y, axis=1), MIN_SS).astype(np.float32)
    ny = np.sqrt(ssy)
    f = np.where(ny > MAXNORM, MAXNORM / ny, 1.0).astype(np.float32)
    y = y * f[:, None]
    return y.reshape(int(batch_size), int(max_comments), cfg.HID)


# ---------------- numpy reference (for arbitrary sizes) ----------------

def np_reference(x, src, dst, seg_ids, W1, b1, W2, b2, batch_size, max_comments):
    n = x.shape[0]

    def seg_sum(vals, ids, nseg):
        out = np.zeros((nseg, vals.shape[1]), np.float32)
        np.add.at(out, ids, vals)
        return out

    def rownorm(v):
        return np.sqrt(np.maximum(np.sum(v * v, axis=1, keepdims=True), MIN_SS))

    def proj(v):
        nn = rownorm(v)
        return np.where(nn > MAXNORM, v / nn * MAXNORM, v)

    def logmap0(v):
        nn = rownorm(v)
        arg = np.minimum(nn, 1 - 1e-7)
        return v * np.arctanh(arg) / nn

    def expmap0(v):
        nn = rownorm(v)
        return v * np.tanh(nn) / nn

    h = np.tanh(seg_sum(x[src] @ W1, dst, n) + b1)
    h = np.tanh(seg_sum(h[src] @ W2, dst, n) + b2)
    h = logmap0(proj(h))
    nseg = int(batch_size) * int(max_comments)
    sums = seg_sum(h, seg_ids, nseg)
    counts = np.zeros(nseg, np.float32)
    np.add.at(counts, seg_ids, 1.0)
    agg = sums / np.maximum(counts, 1.0)[:, None]
    agg = proj(expmap0(agg))
    return agg.reshape(int(batch_size), int(max_comments), -1)


# ====================================================================
# Harness entry point: kernel(**inputs) -> np.ndarray
# ====================================================================

_CACHE = {}


def kernel(x, src, dst, seg_ids, W1, b1, W2, b2, batch_size, max_comments):
    """Full-input GNN ComEnc kernel on 8 Trainium2 NeuronCores.

    Accepts the unsharded inputs of reference.setup_inputs() and returns
    the full (batch, max_comments, HID) float32 output.
    """
    from concourse.bass_utils import run_bass_kernel_spmd

    x = np.asarray(x, dtype=np.float32)
    src = np.asarray(src).astype(np.int64)
    dst = np.asarray(dst).astype(np.int64)
    seg_ids = np.asarray(seg_ids).astype(np.int64)
    W1 = np.asarray(W1, dtype=np.float32)
    b1 = np.asarray(b1, dtype=np.float32)
    W2 = np.asarray(W2, dtype=np.float32)
    b2 = np.asarray(b2, dtype=np.float32)
    bs = int(np.asarray(batch_size))
    mc = int(np.asarray(max_comments))

    n_nodes, in_dim = x.shape
    hid = W1.shape[1]
    nseg = bs * mc
    n_cores = 8

    cfg = Cfg(n_nodes, in_dim, hid, nseg, n_cores)
    ntiles, per_core, gpos = host_prep(cfg, src, dst)

    key = (n_nodes, in_dim, hid, nseg, ntiles.tobytes())
    if key in _CACHE:
        nc = _CACHE[key]
    else:
        nc = build(cfg, ntiles)
        _CACHE.clear()
        _CACHE[key] = nc

    maps = host_inputs(cfg, x, seg_ids, W1, b1, W2, b2, per_core, gpos)
    res = run_bass_kernel_spmd(nc, maps, core_ids=list(range(n_cores)))
    partials = [r["pooled"] for r in res.results]
    out = host_epilogue(cfg, partials, bs, mc)
    return np.ascontiguousarray(out.astype(np.float32))



# revision 15
# speedup vs baseline: 1.6723x; 1.1849x over previous
"""GNN message-passing kernel for Trainium2 (8 NeuronCores, SPMD).

Computation (see reference):
  h1 = tanh(segsum(x[src] -> dst) @ W1 + b1)        [uses A(xW) = (Ax)W]
  h2 = tanh(segsum(h1[src] -> dst) @ W2 + b2)
  ht = logmap0(proj(h2))  (rowwise scale)
  pooled = segment mean over seg_ids, then expmap0/proj (host epilogue)

Sharding: nodes split contiguously over cores (dst-shard); within each
shard nodes are re-permuted into (block, slot) positions, balancing the
per-(block, src-class) edge cells to multiples of 128 (overflow
concentrated in 2 blocks/half).  The spmm is a one-hot matmul per
128-edge tile; rows gathered via gpsimd.dma_gather (int16 idx).

The h1 exchange is split into two AllGathers (shard halves, blocks 0-63
and 64-127); AG_A is issued mid-layer-1.  Layer 2 edges are classed by
src location: 0=own shard (local h1_shard, no collective), 1/2=remote
half A (cores 0-3 / 4-7), 3/4=remote half B.  L2 runs in two PSUM
passes (classes 0-2 -> SBUF partial, then 3-4 + combine), so gathers
start the moment L1 ends and AG_B is hidden behind pass-1 work.
"""

import math
from contextlib import ExitStack

import numpy as np
import ml_dtypes

import concourse.bass as bass
import concourse.tile as tile
import concourse.bacc as bacc
from concourse import mybir

BF16 = mybir.dt.bfloat16
F32 = mybir.dt.float32
I16 = mybir.dt.int16
AF = mybir.ActivationFunctionType
ALU = mybir.AluOpType

MAXNORM = 1.0 - 1e-5
MIN_SS = 1e-15

SUB = 3072          # gather indices per dma_gather call (descriptor ring limit)
GRP = 4             # dst blocks (of 128 nodes) per PSUM group
NREG_H = 62         # capped blocks per half-shard (of 64); rest overflow
L2P1 = (0, 1, 2)    # layer-2 pass-1 classes (local + remote half A)
L2P2 = (3, 4)       # layer-2 pass-2 classes (remote half B)


class Cfg:
    def __init__(self, n_nodes, in_dim, hid, n_seg, n_cores):
        self.N = n_nodes
        self.IN = in_dim
        self.HID = hid
        self.NSEG = n_seg
        self.NC = n_cores
        self.SHARD = n_nodes // n_cores
        assert self.SHARD % 128 == 0
        self.NBLK = self.SHARD // 128
        assert self.NBLK % GRP == 0
        self.NGRP = self.NBLK // GRP
        self.CH = min(32768, n_nodes)
        assert n_nodes % self.CH == 0
        self.NCHUNK = n_nodes // self.CH
        self.HB = self.SHARD // 2           # rows per shard half
        self.NSEGCH = (n_seg + 127) // 128


def _balance_half(d, nblk, cap=512, nreg=NREG_H):
    """Assign nodes (rows of d = multi-class in-degree vectors) to nblk
    blocks of 128 slots, keeping per-(block,class) sums <= cap for the
    first nreg blocks; the highest-degree nodes go to the overflow
    blocks first.  Returns pos[n] in [0, nblk*128)."""
    n = d.shape[0]
    tot = d.sum(1)
    order = np.argsort(-tot, kind="stable")
    novf = (nblk - nreg) * 128
    cells = np.zeros((nblk, d.shape[1]), np.int64)
    counts = np.zeros(nblk, np.int64)
    assign = np.empty(n, np.int64)
    for i in range(n):
        v = order[i]
        dv = d[v]
        c2 = cells + dv
        if i < novf:
            cand = np.nonzero(counts[nreg:] < 128)[0] + nreg
        else:
            ok = (counts[:nreg] < 128) & (c2[:nreg] <= cap).all(1)
            cand = np.nonzero(ok)[0]
            if not len(cand):
                cand = np.nonzero(counts < 128)[0]
        b = cand[np.argmin(c2[cand].max(1))]
        assign[v] = b
        counts[b] += 1
        cells[b] += dv
    pos = np.empty(n, np.int64)
    nxt = np.zeros(nblk, np.int64)
    for v in range(n):
        b = assign[v]
        pos[v] = b * 128 + nxt[b]
        nxt[b] += 1
    return pos


def _build_stream(cfg, ntiles, core, g_all, cls, b_all, idxval, slot):
    """Pack edges into the canonical padded (g, cls, b) tile stream.
    Returns per-core list of (idx16 [TOT], slots [TOT])."""
    NC = cfg.NC
    ncls = ntiles.shape[1]
    NTILES = int(ntiles.sum())
    TOT = NTILES * 128
    counts = np.zeros((NC, cfg.NGRP, ncls, GRP), dtype=np.int64)
    np.add.at(counts, (core, g_all, cls, b_all), 1)
    order = np.lexsort((idxval, b_all, cls, g_all, core))
    out = []
    for c in range(NC):
        idx16 = np.zeros(TOT, dtype=np.int64)
        slots = np.full(TOT, -1.0, dtype=np.float32)
        sel = order[core[order] == c]
        ci = idxval[sel]
        cs = slot[sel]
        ccnt = counts[c]
        pos = 0
        ep = 0
        for g in range(cfg.NGRP):
            for k in range(ncls):
                for b in range(GRP):
                    n = int(ccnt[g, k, b])
                    cap = int(ntiles[g, k, b]) * 128
                    assert n <= cap, (g, k, b, n, cap)
                    if n > 0:
                        idx16[pos:pos + n] = ci[ep:ep + n]
                        slots[pos:pos + n] = cs[ep:ep + n]
                        ep += n
                    pos += cap
        assert ep == len(sel)
        out.append((idx16, slots))
    return out


def _wrap_idx(a):
    iw = a.reshape(-1, 16).T                # [16, TOT/16]
    return np.tile(iw, (8, 1)).astype(np.int16)  # [128, TOT/16]


def host_prep(cfg, src, dst):
    """Balanced permutation + per-layer edge tile streams.

    Returns (ntiles1, ntiles2, per_core dicts, gpos)."""
    NC, SH, CH = cfg.NC, cfg.SHARD, cfg.CH
    src = np.asarray(src).astype(np.int64)
    dst = np.asarray(dst).astype(np.int64)

    chunk1 = src // CH                      # L1 class: x-table chunk
    s_core = src // SH
    s_rng = s_core // 4
    s_halfn = (src % SH) // cfg.HB          # half by node id (pi-invariant)

    # ---- balanced node -> (block, slot) permutation per core+half ----
    gpos = np.empty(cfg.N, np.int64)
    for c in range(NC):
        lo, hi = c * SH, (c + 1) * SH
        m = (dst >= lo) & (dst < hi)
        u = src[m]
        ld = dst[m] - lo
        cls2_c = np.where(u // SH == c, 0,
                          1 + (u // SH) // 4 + 2 * ((u % SH) // cfg.HB))
        d9 = np.zeros((SH, 9), np.int64)
        np.add.at(d9, (ld, u // CH), 1)                 # cols 0..3: L1
        np.add.at(d9, (ld, 4 + cls2_c), 1)              # cols 4..8: L2
        node_half = (np.arange(SH) // cfg.HB)
        for h in range(2):
            vs = np.nonzero(node_half == h)[0]
            p = _balance_half(d9[vs], cfg.NBLK // 2)
            gpos[lo + vs] = lo + h * cfg.HB + p

    pdst = gpos[dst]
    core = pdst // SH
    blk = (pdst % SH) // 128
    slot = pdst % 128
    g_all = blk // GRP
    b_all = blk % GRP

    # ---- layer-1 cells (class = x chunk) ----
    counts1 = np.zeros((NC, cfg.NGRP, cfg.NCHUNK, GRP), dtype=np.int64)
    np.add.at(counts1, (core, g_all, chunk1, b_all), 1)
    ntiles1 = (counts1.max(axis=0) + 127) // 128
    empty = ntiles1.sum(axis=1) == 0
    ntiles1[:, 0, :][empty] = 1
    idx1 = src % CH

    # ---- layer-2 cells (class = src location) ----
    pl = gpos[src] % SH                     # position within src core
    local = s_core == core
    cls2 = np.where(local, 0, 1 + s_rng + 2 * s_halfn)
    base = (s_core - 4 * s_rng) * cfg.HB
    idx2 = np.where(local, pl, base + pl - cfg.HB * s_halfn)
    assert idx2.min() >= 0 and idx2.max() < CH

    counts2 = np.zeros((NC, cfg.NGRP, 5, GRP), dtype=np.int64)
    np.add.at(counts2, (core, g_all, cls2, b_all), 1)
    ntiles2 = (counts2.max(axis=0) + 127) // 128
    p1 = ntiles2[:, 0:3, :].sum(axis=1) == 0
    ntiles2[:, 0, :][p1] = 1
    p2 = ntiles2[:, 3:5, :].sum(axis=1) == 0
    ntiles2[:, 3, :][p2] = 1

    st1 = _build_stream(cfg, ntiles1, core, g_all, chunk1, b_all, idx1, slot)
    st2 = _build_stream(cfg, ntiles2, core, g_all, cls2, b_all, idx2, slot)
    NT1, NT2 = int(ntiles1.sum()), int(ntiles2.sum())
    per_core = []
    for c in range(NC):
        per_core.append({
            "idx16_l1": _wrap_idx(st1[c][0]),
            "idx16_l2": _wrap_idx(st2[c][0]),
            "dstslot1": st1[c][1].reshape(NT1, 128).T.copy().astype(np.float32),
            "dstslot2": st2[c][1].reshape(NT2, 128).T.copy().astype(np.float32),
        })
    return ntiles1, ntiles2, per_core, gpos


def _mm_schedule(cfg, ntiles, passes):
    """sched[g][cls] = [(tile_global_col, block b, start, stop)]; start/stop
    flags close each accumulation within its pass."""
    ncls = ntiles.shape[1]
    sched = [[[] for _ in range(ncls)] for _ in range(cfg.NGRP)]
    tcol = 0
    for g in range(cfg.NGRP):
        for pa in passes:
            tot_b = ntiles[g][list(pa)].sum(axis=0)   # [GRP]
            seen_b = np.zeros(GRP, dtype=np.int64)
            for k in pa:
                for b in range(GRP):
                    for _ in range(int(ntiles[g, k, b])):
                        start = seen_b[b] == 0
                        stop = seen_b[b] == tot_b[b] - 1
                        sched[g][k].append((tcol, b, bool(start), bool(stop)))
                        seen_b[b] += 1
                        tcol += 1
    return sched


def build(cfg, ntiles1, ntiles2, debug_taps=False):
    """Build the Bass program. Returns nc."""
    N, IN, HID = cfg.N, cfg.IN, cfg.HID
    NT1, NT2 = int(ntiles1.sum()), int(ntiles2.sum())
    sched1 = _mm_schedule(cfg, ntiles1, [list(range(cfg.NCHUNK))])
    sched2 = _mm_schedule(cfg, ntiles2, [list(L2P1), list(L2P2)])

    nc = bacc.Bacc("TRN2", target_bir_lowering=False,
                   dynamic_dma_scratch_size=65536)

    x_d = nc.dram_tensor("x_bf16", [N, IN], BF16, kind="ExternalInput")
    idx1_d = nc.dram_tensor("idx16_l1", [128, NT1 * 8], I16, kind="ExternalInput")
    idx2_d = nc.dram_tensor("idx16_l2", [128, NT2 * 8], I16, kind="ExternalInput")
    slot1_d = nc.dram_tensor("dstslot1", [128, NT1], F32, kind="ExternalInput")
    slot2_d = nc.dram_tensor("dstslot2", [128, NT2], F32, kind="ExternalInput")
    segid_d = nc.dram_tensor("segid", [128, cfg.NBLK], F32, kind="ExternalInput")
    iota_d = nc.dram_tensor("iota128", [128, 128], BF16, kind="ExternalInput")
    iotas_d = nc.dram_tensor("iota_seg", [128, cfg.NSEGCH * 128], F32, kind="ExternalInput")
    ident_d = nc.dram_tensor("ident", [128, 128], BF16, kind="ExternalInput")
    w1_d = nc.dram_tensor("W1", [IN, HID], BF16, kind="ExternalInput")
    w2_d = nc.dram_tensor("W2", [HID, HID], BF16, kind="ExternalInput")
    b1_d = nc.dram_tensor("b1rep", [128, HID], F32, kind="ExternalInput")
    b2_d = nc.dram_tensor("b2rep", [128, HID], F32, kind="ExternalInput")

    h1_shard = nc.dram_tensor("h1_shard", [cfg.SHARD, HID], BF16)
    h1_ha = nc.dram_tensor("h1_halfA", [cfg.HB * cfg.NC, HID], BF16,
                           addr_space="Shared")
    h1_hb = nc.dram_tensor("h1_halfB", [cfg.HB * cfg.NC, HID], BF16,
                           addr_space="Shared")
    out_d = nc.dram_tensor("pooled", [cfg.NSEGCH * 128, HID + 1], F32,
                           kind="ExternalOutput")

    KIN = IN // 128   # k-chunks for W1 (2)

    with tile.TileContext(nc) as tc, ExitStack() as ctx:
        const = ctx.enter_context(tc.tile_pool(name="const", bufs=1))
        idxp = ctx.enter_context(tc.tile_pool(name="idxp", bufs=4))
        slotp = ctx.enter_context(tc.tile_pool(name="slotp", bufs=3))
        ebufp = ctx.enter_context(tc.tile_pool(name="ebufp", bufs=3))
        sp = ctx.enter_context(tc.tile_pool(name="sp", bufs=4))
        flshp = ctx.enter_context(tc.tile_pool(name="flshp", bufs=3))
        xtp = ctx.enter_context(tc.tile_pool(name="xtp", bufs=4))
        hp = ctx.enter_context(tc.tile_pool(name="hp", bufs=3))
        h2allp = ctx.enter_context(tc.tile_pool(name="h2allp", bufs=1))
        aggp = ctx.enter_context(tc.tile_pool(name="aggp", bufs=1))
        normp = ctx.enter_context(tc.tile_pool(name="normp", bufs=1))
        htp = ctx.enter_context(tc.tile_pool(name="htp", bufs=3))

        ctx_spmm = ctx.enter_context(ExitStack())
        ps_acc = ctx_spmm.enter_context(tc.tile_pool(name="ps_acc", bufs=4, space="PSUM"))
        ps_tr = ctx_spmm.enter_context(tc.tile_pool(name="ps_tr", bufs=1, space="PSUM"))
        ps_h = ctx_spmm.enter_context(tc.tile_pool(name="ps_h", bufs=2, space="PSUM"))

        # ---- constants ----
        iota128 = const.tile([128, 128], BF16)
        nc.sync.dma_start(iota128[:], iota_d[:])
        iotaseg = const.tile([128, cfg.NSEGCH * 128], F32)
        nc.sync.dma_start(iotaseg[:], iotas_d[:])
        ident = const.tile([128, 128], BF16)
        nc.sync.dma_start(ident[:], ident_d[:])
        segid = const.tile([128, cfg.NBLK], F32)
        nc.sync.dma_start(segid[:], segid_d[:])
        w1_sb = [const.tile([128, HID], BF16, tag=f"w1_{k}", name=f"w1_{k}")
                 for k in range(KIN)]
        for k in range(KIN):
            nc.sync.dma_start(w1_sb[k][:], w1_d[k * 128:(k + 1) * 128, :])
        w2_sb = const.tile([128, HID], BF16)
        nc.sync.dma_start(w2_sb[:], w2_d[:])
        b1_sb = const.tile([128, HID], F32)
        nc.sync.dma_start(b1_sb[:], b1_d[:])
        b2_sb = const.tile([128, HID], F32)
        nc.sync.dma_start(b2_sb[:], b2_d[:])

        h2_all = h2allp.tile([128, cfg.NBLK * HID], BF16)
        agg_sb = aggp.tile([128, cfg.NBLK * HID], BF16)
        norms2 = normp.tile([128, cfg.NBLK], F32)
        scale = normp.tile([128, cfg.NBLK], F32)
        na = normp.tile([128, cfg.NBLK], F32)
        nb_t = normp.tile([128, cfg.NBLK], F32)

        def spmm_pass(layer, sched, classes, tables, idx_d, slot_d, feat,
                      out_block, after_group=None):
            """One spmm pass over `classes`.  tables: cls -> DRAM table AP.
            out_block(g, b, agg_ps) consumes the accumulated [128(slot),
            feat] PSUM tile for block nb=g*GRP+b.  One PSUM bank per
            accumulation (start=True clears the whole bank)."""
            for g in range(cfg.NGRP):
                accs = [ps_acc.tile([128, IN], F32, tag="acc", name=f"acc{b}")
                        for b in range(GRP)]

                def acc_slice(b):
                    return accs[b][:, :feat]

                for k in classes:
                    tiles_k = sched[g][k]
                    if not tiles_k:
                        continue
                    tbl = tables[k]
                    for s0 in range(0, len(tiles_k), SUB // 128):
                        stiles = tiles_k[s0:s0 + SUB // 128]
                        nidx = len(stiles) * 128
                        col0 = stiles[0][0]
                        it = idxp.tile([128, SUB // 16], I16, tag="it")
                        nc.sync.dma_start(
                            it[:, :nidx // 16],
                            idx_d[:, col0 * 8:col0 * 8 + nidx // 16])
                        st = slotp.tile([128, SUB // 128], F32, tag="st")
                        nc.sync.dma_start(
                            st[:, :len(stiles)],
                            slot_d[:, col0:col0 + len(stiles)])
                        eb = ebufp.tile([128, (SUB // 128) * feat], BF16,
                                        tag=f"eb{layer}")
                        nc.gpsimd.dma_gather(
                            out_ap=eb[:, :len(stiles) * feat].rearrange(
                                "p (n f) -> p n f", f=feat),
                            in_ap=tbl,
                            idxs_ap=it[:, :nidx // 16],
                            num_idxs=nidx,
                            num_idxs_reg=nidx,
                            elem_size=feat,
                        )
                        for j, (tcol, b, st_f, sp_f) in enumerate(stiles):
                            s_t = sp.tile([128, 128], BF16, tag="s_t")
                            nc.vector.tensor_scalar(
                                s_t[:], iota128[:], st[:, j:j + 1], None,
                                ALU.is_equal)
                            nc.tensor.matmul(
                                acc_slice(b),
                                s_t[:],
                                eb[:, j * feat:(j + 1) * feat],
                                start=st_f, stop=sp_f,
                            )
                for b in range(GRP):
                    out_block(g, b, acc_slice(b))
                if after_group is not None:
                    after_group(g)

        def l1_block(g, b, agg_ps):
            nb = g * GRP + b
            ax = flshp.tile([128, IN], BF16, tag="ax1")
            nc.scalar.activation(ax[:], agg_ps, AF.Copy)
            h_ps = ps_h.tile([128, HID], F32, tag="hps", name="h_ps")
            for h in range(KIN):
                t_ps = ps_tr.tile([128, 128], BF16, tag="tps")
                nc.tensor.transpose(t_ps[:], ax[:, h * 128:(h + 1) * 128], ident[:])
                xt = xtp.tile([128, 128], BF16, tag="xt")
                nc.scalar.activation(xt[:], t_ps[:], AF.Copy)
                nc.tensor.matmul(h_ps[:], xt[:], w1_sb[h][:],
                                 start=(h == 0), stop=(h == KIN - 1))
            htmp = hp.tile([128, HID], F32, tag="htmp")
            nc.vector.tensor_add(htmp[:], h_ps[:], b1_sb[:])
            h1b = hp.tile([128, HID], BF16, tag="h1b")
            nc.scalar.activation(h1b[:], htmp[:], AF.Tanh)
            nc.sync.dma_start(h1_shard[nb * 128:(nb + 1) * 128, :], h1b[:])

        def ag_hook(g):
            if g == cfg.NGRP // 2 - 1:
                nc.gpsimd.collective_compute(
                    "AllGather", ALU.bypass,
                    ins=[h1_shard[0:cfg.HB, :].opt()],
                    outs=[h1_ha.ap().opt()],
                    replica_groups=[list(range(cfg.NC))],
                )
            if g == cfg.NGRP - 1:
                nc.gpsimd.collective_compute(
                    "AllGather", ALU.bypass,
                    ins=[h1_shard[cfg.HB:cfg.SHARD, :].opt()],
                    outs=[h1_hb.ap().opt()],
                    replica_groups=[list(range(cfg.NC))],
                )

        def flush_block(g, b, agg_ps):
            nb = g * GRP + b
            nc.scalar.activation(agg_sb[:, nb * HID:(nb + 1) * HID], agg_ps,
                                 AF.Copy)

        def l2_final(g, b, agg_ps):
            nb = g * GRP + b
            a2 = flshp.tile([128, HID], BF16, tag="a22")
            nc.vector.tensor_add(a2[:], agg_ps,
                                 agg_sb[:, nb * HID:(nb + 1) * HID])
            t_ps = ps_tr.tile([128, 128], BF16, tag="tps")
            nc.tensor.transpose(t_ps[:], a2[:], ident[:])
            a2t = xtp.tile([128, 128], BF16, tag="xt")
            nc.scalar.activation(a2t[:], t_ps[:], AF.Copy)
            h_ps = ps_h.tile([128, HID], F32, tag="hps", name="h_ps")
            nc.tensor.matmul(h_ps[:], a2t[:], w2_sb[:], start=True, stop=True)
            htmp = hp.tile([128, HID], F32, tag="htmp")
            nc.vector.tensor_add(htmp[:], h_ps[:], b2_sb[:])
            nc.scalar.activation(h2_all[:, nb * HID:(nb + 1) * HID], htmp[:],
                                 AF.Tanh)

        # ---------------- layer 1 (+ split AllGather issue) ----------------
        xtabs = {k: x_d[k * cfg.CH:(k + 1) * cfg.CH, :]
                 for k in range(cfg.NCHUNK)}
        spmm_pass(1, sched1, list(range(cfg.NCHUNK)), xtabs, idx1_d, slot1_d,
                  IN, l1_block, after_group=ag_hook)

        # ---------------- layer 2, pass 1 (local + half A) ----------------
        t2a = {0: h1_shard.ap(),
               1: h1_ha[0:cfg.CH, :],
               2: h1_ha[cfg.CH:2 * cfg.CH, :]}
        spmm_pass(2, sched2, list(L2P1), t2a, idx2_d, slot2_d, HID,
                  flush_block)

        # ---------------- layer 2, pass 2 (half B + combine) ----------------
        t2b = {3: h1_hb[0:cfg.CH, :],
               4: h1_hb[cfg.CH:2 * cfg.CH, :]}
        spmm_pass(2, sched2, list(L2P2), t2b, idx2_d, slot2_d, HID,
                  l2_final)

        # ---------------- norms + logmap scale ----------------
        for nbk in range(cfg.NBLK):
            h2b = h2_all[:, nbk * HID:(nbk + 1) * HID]
            sq = htp.tile([128, HID], F32, tag="sq")
            nc.vector.tensor_mul(sq[:], h2b, h2b)
            nc.vector.tensor_reduce(norms2[:, nbk:nbk + 1], sq[:],
                                    mybir.AxisListType.X, ALU.add)
        # norm = sqrt(max(ss, MIN_SS)); nclip = min(norm, MAXNORM)
        nc.vector.tensor_scalar_max(na[:], norms2[:], MIN_SS)
        nc.scalar.activation(nb_t[:], na[:], AF.Sqrt)        # nb_t = norm
        nc.vector.tensor_scalar_min(na[:], nb_t[:], MAXNORM)  # na = nclip
        # artanh(nclip) = 0.5*ln((1+n)/(1-n)); scale = artanh/norm
        one_m = normp.tile([128, cfg.NBLK], F32)
        nc.vector.tensor_scalar(one_m[:], na[:], -1.0, 1.0, ALU.mult, ALU.add)
        one_p = normp.tile([128, cfg.NBLK], F32)
        nc.vector.tensor_scalar_add(one_p[:], na[:], 1.0)
        rcp = normp.tile([128, cfg.NBLK], F32)
        nc.vector.reciprocal(rcp[:], one_m[:])
        rat = normp.tile([128, cfg.NBLK], F32)
        nc.vector.tensor_mul(rat[:], one_p[:], rcp[:])
        lg = normp.tile([128, cfg.NBLK], F32)
        nc.scalar.activation(lg[:], rat[:], AF.Ln)
        nc.vector.tensor_scalar_mul(lg[:], lg[:], 0.5)
        rcpn = normp.tile([128, cfg.NBLK], F32)
        nc.vector.reciprocal(rcpn[:], nb_t[:])
        nc.vector.tensor_mul(scale[:], lg[:], rcpn[:])

        # ---------------- pooling ----------------
        ctx_spmm.close()
        ps_pool = ctx.enter_context(
            tc.tile_pool(name="ps_pool", bufs=max(cfg.NSEGCH, 1), space="PSUM"))
        pool_ps = [ps_pool.tile([128, HID + 1], F32, tag="pool", name=f"pool{sc}")
                   for sc in range(cfg.NSEGCH)]
        for nbk in range(cfg.NBLK):
            h2b = h2_all[:, nbk * HID:(nbk + 1) * HID]
            ht = htp.tile([128, HID + 1], BF16, tag="ht")
            nc.vector.tensor_scalar(ht[:, :HID], h2b, scale[:, nbk:nbk + 1],
                                    None, ALU.mult)
            nc.vector.memset(ht[:, HID:HID + 1], 1.0)
            for sc in range(cfg.NSEGCH):
                sg = sp.tile([128, 128], BF16, tag="sg")
                nc.vector.tensor_scalar(
                    sg[:], iotaseg[:, sc * 128:(sc + 1) * 128],
                    segid[:, nbk:nbk + 1], None, ALU.is_equal)
                nc.tensor.matmul(
                    pool_ps[sc][:], sg[:], ht[:],
                    start=(nbk == 0), stop=(nbk == cfg.NBLK - 1))
        for sc in range(cfg.NSEGCH):
            po = htp.tile([128, HID + 1], F32, tag="po")
            nc.vector.tensor_copy(po[:], pool_ps[sc][:])
            nc.sync.dma_start(out_d[sc * 128:(sc + 1) * 128, :], po[:])

    nc.compile()
    return nc


def host_inputs(cfg, x, seg_ids, W1, b1, W2, b2, per_core, gpos):
    """Per-core in_maps for run_bass_kernel_spmd."""
    N, IN, HID = cfg.N, cfg.IN, cfg.HID
    x_bf16 = np.ascontiguousarray(x.astype(ml_dtypes.bfloat16))
    iota128 = np.tile(np.arange(128, dtype=np.float32), (128, 1)).astype(ml_dtypes.bfloat16)
    iotaseg = np.tile(np.arange(cfg.NSEGCH * 128, dtype=np.float32), (128, 1))
    ident = np.eye(128, dtype=np.float32).astype(ml_dtypes.bfloat16)
    w1 = np.ascontiguousarray(W1.astype(ml_dtypes.bfloat16))
    w2 = np.ascontiguousarray(W2.astype(ml_dtypes.bfloat16))
    b1r = np.tile(np.asarray(b1, np.float32), (128, 1))
    b2r = np.tile(np.asarray(b2, np.float32), (128, 1))
    seg_pos = np.empty(cfg.N, np.float32)
    seg_pos[gpos] = np.asarray(seg_ids, np.float32)   # seg id by position
    maps = []
    for c in range(cfg.NC):
        segc = seg_pos[c * cfg.SHARD:(c + 1) * cfg.SHARD].reshape(cfg.NBLK, 128).T
        maps.append({
            "x_bf16": x_bf16,
            "idx16_l1": per_core[c]["idx16_l1"],
            "idx16_l2": per_core[c]["idx16_l2"],
            "dstslot1": per_core[c]["dstslot1"],
            "dstslot2": per_core[c]["dstslot2"],
            "segid": np.ascontiguousarray(segc),
            "iota128": iota128,
            "iota_seg": np.ascontiguousarray(iotaseg.astype(np.float32)),
            "ident": ident,
            "W1": w1,
            "W2": w2,
            "b1rep": b1r,
            "b2rep": b2r,
        })
    return maps


def host_epilogue(cfg, partials, batch_size, max_comments):
    """partials: list of per-core [NSEGCH*128, HID+1] f32."""
    acc = np.zeros_like(partials[0], dtype=np.float64)
    for p in partials:
        acc += p.astype(np.float64)
    acc = acc.astype(np.float32)
    nseg = cfg.NSEG
    sums = acc[:nseg, :cfg.HID]
    counts = acc[:nseg, cfg.HID]
    agg = sums / np.maximum(counts, 1.0)[:, None]
    # expmap0 then proj
    ss = np.maximum(np.sum(agg * agg, axis=1), MIN_SS).astype(np.float32)
    norm = np.sqrt(ss)
    y = agg * (np.tanh(norm) / norm)[:, None]
    ssy = np.maximum(np.sum(y * y, axis=1), MIN_SS).astype(np.float32)
    ny = np.sqrt(ssy)
    f = np.where(ny > MAXNORM, MAXNORM / ny, 1.0).astype(np.float32)
    y = y * f[:, None]
    return y.reshape(int(batch_size), int(max_comments), cfg.HID)


# ====================================================================
# Harness entry point: kernel(**inputs) -> np.ndarray
# ====================================================================

_CACHE = {}


def kernel(x, src, dst, seg_ids, W1, b1, W2, b2, batch_size, max_comments):
    """Full-input GNN ComEnc kernel on 8 Trainium2 NeuronCores.

    Accepts the unsharded inputs of reference.setup_inputs() and returns
    the full (batch, max_comments, HID) float32 output.
    """
    from concourse.bass_utils import run_bass_kernel_spmd

    x = np.asarray(x, dtype=np.float32)
    src = np.asarray(src).astype(np.int64)
    dst = np.asarray(dst).astype(np.int64)
    seg_ids = np.asarray(seg_ids).astype(np.int64)
    W1 = np.asarray(W1, dtype=np.float32)
    b1 = np.asarray(b1, dtype=np.float32)
    W2 = np.asarray(W2, dtype=np.float32)
    b2 = np.asarray(b2, dtype=np.float32)
    bs = int(np.asarray(batch_size))
    mc = int(np.asarray(max_comments))

    n_nodes, in_dim = x.shape
    hid = W1.shape[1]
    nseg = bs * mc
    n_cores = 8

    cfg = Cfg(n_nodes, in_dim, hid, nseg, n_cores)
    ntiles1, ntiles2, per_core, gpos = host_prep(cfg, src, dst)

    key = (n_nodes, in_dim, hid, nseg, ntiles1.tobytes(), ntiles2.tobytes())
    if key in _CACHE:
        nc = _CACHE[key]
    else:
        nc = build(cfg, ntiles1, ntiles2)
        _CACHE.clear()
        _CACHE[key] = nc

    maps = host_inputs(cfg, x, seg_ids, W1, b1, W2, b2, per_core, gpos)
    res = run_bass_kernel_spmd(nc, maps, core_ids=list(range(n_cores)))
    partials = [r["pooled"] for r in res.results]
    out = host_epilogue(cfg, partials, bs, mc)
    return np.ascontiguousarray(out.astype(np.float32))


# revision 20
# speedup vs baseline: 1.7141x; 1.0250x over previous
"""GNN message-passing kernel for Trainium2 (8 NeuronCores, SPMD).

Computation (see reference):
  h1 = tanh(segsum(x[src] -> dst) @ W1 + b1)        [uses A(xW) = (Ax)W]
  h2 = tanh(segsum(h1[src] -> dst) @ W2 + b2)
  ht = logmap0(proj(h2))  (rowwise scale)
  pooled = segment mean over seg_ids, then expmap0/proj (host epilogue)

Sharding: nodes split contiguously over cores (dst-shard); within each
shard nodes are re-permuted into (block, slot) positions, balancing the
per-(block, src-class) edge cells to multiples of 128 (overflow
concentrated in 2 blocks/half).  The spmm is a one-hot matmul per
128-edge tile; rows gathered via gpsimd.dma_gather (int16 idx).

The h1 exchange is split into two AllGathers (shard halves, blocks 0-63
and 64-127); AG_A is issued mid-layer-1.  Layer 2 edges are classed by
src location: 0=own shard (local h1_shard, no collective), 1/2=remote
half A (cores 0-3 / 4-7), 3/4=remote half B.  L2 runs in two PSUM
passes (classes 0-2 -> SBUF partial, then 3-4 + combine), so gathers
start the moment L1 ends and AG_B is hidden behind pass-1 work.
"""

import math
from contextlib import ExitStack

import numpy as np
import ml_dtypes

import concourse.bass as bass
import concourse.tile as tile
import concourse.bacc as bacc
from concourse import mybir

BF16 = mybir.dt.bfloat16
F32 = mybir.dt.float32
I16 = mybir.dt.int16
AF = mybir.ActivationFunctionType
ALU = mybir.AluOpType

MAXNORM = 1.0 - 1e-5
MIN_SS = 1e-15

SUB = 3072          # gather indices per dma_gather call (descriptor ring limit)
GRP = 4             # dst blocks (of 128 nodes) per PSUM group
NREG_H = 62         # capped blocks per half-shard (of 64); rest overflow
L2P1 = (0, 1, 2)    # layer-2 pass-1 classes (local + remote half A)
L2P2 = (3, 4)       # layer-2 pass-2 classes (remote half B)


class Cfg:
    def __init__(self, n_nodes, in_dim, hid, n_seg, n_cores):
        self.N = n_nodes
        self.IN = in_dim
        self.HID = hid
        self.NSEG = n_seg
        self.NC = n_cores
        self.SHARD = n_nodes // n_cores
        assert self.SHARD % 128 == 0
        self.NBLK = self.SHARD // 128
        assert self.NBLK % GRP == 0
        self.NGRP = self.NBLK // GRP
        self.CH = min(32768, n_nodes)
        assert n_nodes % self.CH == 0
        self.NCHUNK = n_nodes // self.CH
        self.HB = self.SHARD // 2           # rows per shard half
        self.NSEGCH = (n_seg + 127) // 128


def _balance_half(d, nblk, cap=512, nreg=NREG_H):
    """Assign nodes (rows of d = multi-class in-degree vectors) to nblk
    blocks of 128 slots, keeping per-(block,class) sums <= cap for the
    first nreg blocks; the highest-degree nodes go to the overflow
    blocks first.  Returns pos[n] in [0, nblk*128)."""
    n = d.shape[0]
    tot = d.sum(1)
    order = np.argsort(-tot, kind="stable")
    novf = (nblk - nreg) * 128
    cells = np.zeros((nblk, d.shape[1]), np.int64)
    counts = np.zeros(nblk, np.int64)
    assign = np.empty(n, np.int64)
    for i in range(n):
        v = order[i]
        dv = d[v]
        c2 = cells + dv
        if i < novf:
            cand = np.nonzero(counts[nreg:] < 128)[0] + nreg
        else:
            ok = (counts[:nreg] < 128) & (c2[:nreg] <= cap).all(1)
            cand = np.nonzero(ok)[0]
            if not len(cand):
                cand = np.nonzero(counts < 128)[0]
        b = cand[np.argmin(c2[cand].max(1))]
        assign[v] = b
        counts[b] += 1
        cells[b] += dv
    pos = np.empty(n, np.int64)
    nxt = np.zeros(nblk, np.int64)
    for v in range(n):
        b = assign[v]
        pos[v] = b * 128 + nxt[b]
        nxt[b] += 1
    return pos


def _build_stream(cfg, ntiles, core, g_all, cls, b_all, idxval, slot):
    """Pack edges into the canonical padded (g, cls, b) tile stream.
    Returns per-core list of (idx16 [TOT], slots [TOT])."""
    NC = cfg.NC
    ncls = ntiles.shape[1]
    NTILES = int(ntiles.sum())
    TOT = NTILES * 128
    counts = np.zeros((NC, cfg.NGRP, ncls, GRP), dtype=np.int64)
    np.add.at(counts, (core, g_all, cls, b_all), 1)
    order = np.lexsort((idxval, b_all, cls, g_all, core))
    out = []
    for c in range(NC):
        idx16 = np.zeros(TOT, dtype=np.int64)
        slots = np.full(TOT, -1.0, dtype=np.float32)
        sel = order[core[order] == c]
        ci = idxval[sel]
        cs = slot[sel]
        ccnt = counts[c]
        pos = 0
        ep = 0
        for g in range(cfg.NGRP):
            for k in range(ncls):
                for b in range(GRP):
                    n = int(ccnt[g, k, b])
                    cap = int(ntiles[g, k, b]) * 128
                    assert n <= cap, (g, k, b, n, cap)
                    if n > 0:
                        idx16[pos:pos + n] = ci[ep:ep + n]
                        slots[pos:pos + n] = cs[ep:ep + n]
                        ep += n
                    pos += cap
        assert ep == len(sel)
        out.append((idx16, slots))
    return out


def _wrap_idx(a):
    iw = a.reshape(-1, 16).T                # [16, TOT/16]
    return np.tile(iw, (8, 1)).astype(np.int16)  # [128, TOT/16]


def host_prep(cfg, src, dst):
    """Balanced permutation + per-layer edge tile streams.

    Returns (ntiles1, ntiles2, per_core dicts, gpos)."""
    NC, SH, CH = cfg.NC, cfg.SHARD, cfg.CH
    src = np.asarray(src).astype(np.int64)
    dst = np.asarray(dst).astype(np.int64)

    chunk1 = src // CH                      # L1 class: x-table chunk
    s_core = src // SH
    s_rng = s_core // 4
    s_halfn = (src % SH) // cfg.HB          # half by node id (pi-invariant)

    # ---- balanced node -> (block, slot) permutation per core+half ----
    gpos = np.empty(cfg.N, np.int64)
    for c in range(NC):
        lo, hi = c * SH, (c + 1) * SH
        m = (dst >= lo) & (dst < hi)
        u = src[m]
        ld = dst[m] - lo
        cls2_c = np.where(u // SH == c, 0,
                          1 + (u // SH) // 4 + 2 * ((u % SH) // cfg.HB))
        d9 = np.zeros((SH, 9), np.int64)
        np.add.at(d9, (ld, u // CH), 1)                 # cols 0..3: L1
        np.add.at(d9, (ld, 4 + cls2_c), 1)              # cols 4..8: L2
        node_half = (np.arange(SH) // cfg.HB)
        for h in range(2):
            vs = np.nonzero(node_half == h)[0]
            p = _balance_half(d9[vs], cfg.NBLK // 2)
            gpos[lo + vs] = lo + h * cfg.HB + p

    pdst = gpos[dst]
    core = pdst // SH
    blk = (pdst % SH) // 128
    slot = pdst % 128
    g_all = blk // GRP
    b_all = blk % GRP

    # ---- layer-1 cells (class = x chunk) ----
    counts1 = np.zeros((NC, cfg.NGRP, cfg.NCHUNK, GRP), dtype=np.int64)
    np.add.at(counts1, (core, g_all, chunk1, b_all), 1)
    ntiles1 = (counts1.max(axis=0) + 127) // 128
    empty = ntiles1.sum(axis=1) == 0
    ntiles1[:, 0, :][empty] = 1
    idx1 = src % CH

    # ---- layer-2 cells (class = src location) ----
    pl = gpos[src] % SH                     # position within src core
    local = s_core == core
    cls2 = np.where(local, 0, 1 + s_rng + 2 * s_halfn)
    base = (s_core - 4 * s_rng) * cfg.HB
    idx2 = np.where(local, pl, base + pl - cfg.HB * s_halfn)
    assert idx2.min() >= 0 and idx2.max() < CH

    counts2 = np.zeros((NC, cfg.NGRP, 5, GRP), dtype=np.int64)
    np.add.at(counts2, (core, g_all, cls2, b_all), 1)
    ntiles2 = (counts2.max(axis=0) + 127) // 128
    p1 = ntiles2[:, 0:3, :].sum(axis=1) == 0
    ntiles2[:, 0, :][p1] = 1
    p2 = ntiles2[:, 3:5, :].sum(axis=1) == 0
    ntiles2[:, 3, :][p2] = 1

    st1 = _build_stream(cfg, ntiles1, core, g_all, chunk1, b_all, idx1, slot)
    st2 = _build_stream(cfg, ntiles2, core, g_all, cls2, b_all, idx2, slot)
    NT1, NT2 = int(ntiles1.sum()), int(ntiles2.sum())
    per_core = []
    for c in range(NC):
        per_core.append({
            "idx16_l1": _wrap_idx(st1[c][0]),
            "idx16_l2": _wrap_idx(st2[c][0]),
            "dstslot1": st1[c][1].reshape(NT1, 128).T.copy().astype(np.float32),
            "dstslot2": st2[c][1].reshape(NT2, 128).T.copy().astype(np.float32),
        })
    return ntiles1, ntiles2, per_core, gpos


def _mm_schedule(cfg, ntiles, passes):
    """sched[g][cls] = [(tile_global_col, block b, start, stop)]; start/stop
    flags close each accumulation within its pass."""
    ncls = ntiles.shape[1]
    sched = [[[] for _ in range(ncls)] for _ in range(cfg.NGRP)]
    tcol = 0
    for g in range(cfg.NGRP):
        for pa in passes:
            tot_b = ntiles[g][list(pa)].sum(axis=0)   # [GRP]
            seen_b = np.zeros(GRP, dtype=np.int64)
            for k in pa:
                for b in range(GRP):
                    for _ in range(int(ntiles[g, k, b])):
                        start = seen_b[b] == 0
                        stop = seen_b[b] == tot_b[b] - 1
                        sched[g][k].append((tcol, b, bool(start), bool(stop)))
                        seen_b[b] += 1
                        tcol += 1
    return sched


def build(cfg, ntiles1, ntiles2, debug_taps=False):
    """Build the Bass program. Returns nc."""
    N, IN, HID = cfg.N, cfg.IN, cfg.HID
    NT1, NT2 = int(ntiles1.sum()), int(ntiles2.sum())
    sched1 = _mm_schedule(cfg, ntiles1, [list(range(cfg.NCHUNK))])
    sched2 = _mm_schedule(cfg, ntiles2, [list(L2P1), list(L2P2)])

    nc = bacc.Bacc("TRN2", target_bir_lowering=False,
                   dynamic_dma_scratch_size=65536)

    x_d = nc.dram_tensor("x_bf16", [N, IN], BF16, kind="ExternalInput")
    idx1_d = nc.dram_tensor("idx16_l1", [128, NT1 * 8], I16, kind="ExternalInput")
    idx2_d = nc.dram_tensor("idx16_l2", [128, NT2 * 8], I16, kind="ExternalInput")
    slot1_d = nc.dram_tensor("dstslot1", [128, NT1], F32, kind="ExternalInput")
    slot2_d = nc.dram_tensor("dstslot2", [128, NT2], F32, kind="ExternalInput")
    segid_d = nc.dram_tensor("segid", [128, cfg.NBLK], F32, kind="ExternalInput")
    iota_d = nc.dram_tensor("iota128", [128, 128], BF16, kind="ExternalInput")
    iotas_d = nc.dram_tensor("iota_seg", [128, cfg.NSEGCH * 128], F32, kind="ExternalInput")
    ident_d = nc.dram_tensor("ident", [128, 128], BF16, kind="ExternalInput")
    w1_d = nc.dram_tensor("W1", [IN, HID], BF16, kind="ExternalInput")
    w2_d = nc.dram_tensor("W2", [HID, HID], BF16, kind="ExternalInput")
    b1_d = nc.dram_tensor("b1rep", [128, HID], F32, kind="ExternalInput")
    b2_d = nc.dram_tensor("b2rep", [128, HID], F32, kind="ExternalInput")

    h1_shard = nc.dram_tensor("h1_shard", [cfg.SHARD, HID], BF16)
    h1_ha = nc.dram_tensor("h1_halfA", [cfg.HB * cfg.NC, HID], BF16,
                           addr_space="Shared")
    h1_hb = nc.dram_tensor("h1_halfB", [cfg.HB * cfg.NC, HID], BF16,
                           addr_space="Shared")
    out_d = nc.dram_tensor("pooled", [cfg.NSEGCH * 128, HID + 1], F32,
                           kind="ExternalOutput")

    KIN = IN // 128   # k-chunks for W1 (2)

    with tile.TileContext(nc) as tc, ExitStack() as ctx:
        const = ctx.enter_context(tc.tile_pool(name="const", bufs=1))
        idxp = ctx.enter_context(tc.tile_pool(name="idxp", bufs=4))
        slotp = ctx.enter_context(tc.tile_pool(name="slotp", bufs=3))
        ebufp = ctx.enter_context(tc.tile_pool(name="ebufp", bufs=3))
        sp = ctx.enter_context(tc.tile_pool(name="sp", bufs=4))
        flshp = ctx.enter_context(tc.tile_pool(name="flshp", bufs=3))
        xtp = ctx.enter_context(tc.tile_pool(name="xtp", bufs=4))
        hp = ctx.enter_context(tc.tile_pool(name="hp", bufs=3))
        h2allp = ctx.enter_context(tc.tile_pool(name="h2allp", bufs=1))
        aggp = ctx.enter_context(tc.tile_pool(name="aggp", bufs=1))
        normp = ctx.enter_context(tc.tile_pool(name="normp", bufs=1))
        htp = ctx.enter_context(tc.tile_pool(name="htp", bufs=3))

        ps_acc = ctx.enter_context(tc.tile_pool(name="ps_acc", bufs=4, space="PSUM"))
        ps_tr = ctx.enter_context(tc.tile_pool(name="ps_tr", bufs=1, space="PSUM"))
        ctx_l1 = ctx.enter_context(ExitStack())
        ps_h = ctx_l1.enter_context(tc.tile_pool(name="ps_h", bufs=2, space="PSUM"))

        # ---- constants ----
        iota128 = const.tile([128, 128], BF16)
        nc.sync.dma_start(iota128[:], iota_d[:])
        iotaseg = const.tile([128, cfg.NSEGCH * 128], F32)
        nc.sync.dma_start(iotaseg[:], iotas_d[:])
        ident = const.tile([128, 128], BF16)
        nc.sync.dma_start(ident[:], ident_d[:])
        segid = const.tile([128, cfg.NBLK], F32)
        nc.sync.dma_start(segid[:], segid_d[:])
        w1_sb = [const.tile([128, HID], BF16, tag=f"w1_{k}", name=f"w1_{k}")
                 for k in range(KIN)]
        for k in range(KIN):
            nc.sync.dma_start(w1_sb[k][:], w1_d[k * 128:(k + 1) * 128, :])
        w2_sb = const.tile([128, HID], BF16)
        nc.sync.dma_start(w2_sb[:], w2_d[:])
        b1_sb = const.tile([128, HID], F32)
        nc.sync.dma_start(b1_sb[:], b1_d[:])
        b2_sb = const.tile([128, HID], F32)
        nc.sync.dma_start(b2_sb[:], b2_d[:])

        h2_all = h2allp.tile([128, cfg.NBLK * HID], BF16)
        agg_sb = aggp.tile([128, cfg.NBLK * HID], BF16)
        norms2 = normp.tile([128, cfg.NBLK], F32)
        scale = normp.tile([128, cfg.NBLK], F32)
        na = normp.tile([128, cfg.NBLK], F32)
        nb_t = normp.tile([128, cfg.NBLK], F32)
        one_m = normp.tile([128, cfg.NBLK], F32)
        one_p = normp.tile([128, cfg.NBLK], F32)
        rcp = normp.tile([128, cfg.NBLK], F32)
        rat = normp.tile([128, cfg.NBLK], F32)
        lg = normp.tile([128, cfg.NBLK], F32)
        rcpn = normp.tile([128, cfg.NBLK], F32)

        def spmm_pass(layer, sched, classes, tables, idx_d, slot_d, feat,
                      out_block, after_group=None):
            """One spmm pass over `classes`.  tables: cls -> DRAM table AP.
            out_block(g, b, agg_ps) consumes the accumulated [128(slot),
            feat] PSUM tile for block nb=g*GRP+b.  One PSUM bank per
            accumulation (start=True clears the whole bank)."""
            for g in range(cfg.NGRP):
                accs = [ps_acc.tile([128, IN], F32, tag="acc", name=f"acc{b}")
                        for b in range(GRP)]

                def acc_slice(b):
                    return accs[b][:, :feat]

                for k in classes:
                    tiles_k = sched[g][k]
                    if not tiles_k:
                        continue
                    tbl = tables[k]
                    for s0 in range(0, len(tiles_k), SUB // 128):
                        stiles = tiles_k[s0:s0 + SUB // 128]
                        nidx = len(stiles) * 128
                        col0 = stiles[0][0]
                        it = idxp.tile([128, SUB // 16], I16, tag="it")
                        nc.sync.dma_start(
                            it[:, :nidx // 16],
                            idx_d[:, col0 * 8:col0 * 8 + nidx // 16])
                        st = slotp.tile([128, SUB // 128], F32, tag="st")
                        nc.sync.dma_start(
                            st[:, :len(stiles)],
                            slot_d[:, col0:col0 + len(stiles)])
                        eb = ebufp.tile([128, (SUB // 128) * feat], BF16,
                                        tag=f"eb{layer}")
                        nc.gpsimd.dma_gather(
                            out_ap=eb[:, :len(stiles) * feat].rearrange(
                                "p (n f) -> p n f", f=feat),
                            in_ap=tbl,
                            idxs_ap=it[:, :nidx // 16],
                            num_idxs=nidx,
                            num_idxs_reg=nidx,
                            elem_size=feat,
                        )
                        for j, (tcol, b, st_f, sp_f) in enumerate(stiles):
                            s_t = sp.tile([128, 128], BF16, tag="s_t")
                            nc.vector.tensor_scalar(
                                s_t[:], iota128[:], st[:, j:j + 1], None,
                                ALU.is_equal)
                            nc.tensor.matmul(
                                acc_slice(b),
                                s_t[:],
                                eb[:, j * feat:(j + 1) * feat],
                                start=st_f, stop=sp_f,
                            )
                for b in range(GRP):
                    out_block(g, b, acc_slice(b))
                if after_group is not None:
                    after_group(g)

        def l1_block(g, b, agg_ps):
            nb = g * GRP + b
            ax = flshp.tile([128, IN], BF16, tag="ax1")
            nc.scalar.activation(ax[:], agg_ps, AF.Copy)
            h_ps = ps_h.tile([128, HID], F32, tag="hps", name="h_ps")
            for h in range(KIN):
                t_ps = ps_tr.tile([128, 128], BF16, tag="tps")
                nc.tensor.transpose(t_ps[:], ax[:, h * 128:(h + 1) * 128], ident[:])
                xt = xtp.tile([128, 128], BF16, tag="xt")
                nc.scalar.activation(xt[:], t_ps[:], AF.Copy)
                nc.tensor.matmul(h_ps[:], xt[:], w1_sb[h][:],
                                 start=(h == 0), stop=(h == KIN - 1))
            htmp = hp.tile([128, HID], F32, tag="htmp")
            nc.vector.tensor_add(htmp[:], h_ps[:], b1_sb[:])
            h1b = hp.tile([128, HID], BF16, tag="h1b")
            nc.scalar.activation(h1b[:], htmp[:], AF.Tanh)
            nc.sync.dma_start(h1_shard[nb * 128:(nb + 1) * 128, :], h1b[:])

        def ag_hook(g):
            if g == cfg.NGRP // 2 - 1:
                nc.gpsimd.collective_compute(
                    "AllGather", ALU.bypass,
                    ins=[h1_shard[0:cfg.HB, :].opt()],
                    outs=[h1_ha.ap().opt()],
                    replica_groups=[list(range(cfg.NC))],
                )
            if g == cfg.NGRP - 1:
                nc.gpsimd.collective_compute(
                    "AllGather", ALU.bypass,
                    ins=[h1_shard[cfg.HB:cfg.SHARD, :].opt()],
                    outs=[h1_hb.ap().opt()],
                    replica_groups=[list(range(cfg.NC))],
                )

        def flush_block(g, b, agg_ps):
            nb = g * GRP + b
            nc.scalar.activation(agg_sb[:, nb * HID:(nb + 1) * HID], agg_ps,
                                 AF.Copy)

        def l2_final(g, b, agg_ps):
            nb = g * GRP + b
            a2 = flshp.tile([128, HID], BF16, tag="a22")
            nc.vector.tensor_add(a2[:], agg_ps,
                                 agg_sb[:, nb * HID:(nb + 1) * HID])
            t_ps = ps_tr.tile([128, 128], BF16, tag="tps")
            nc.tensor.transpose(t_ps[:], a2[:], ident[:])
            a2t = xtp.tile([128, 128], BF16, tag="xt")
            nc.scalar.activation(a2t[:], t_ps[:], AF.Copy)
            h_ps = ps_tr.tile([128, HID], F32, tag="hps2", name="h_ps2")
            nc.tensor.matmul(h_ps[:], a2t[:], w2_sb[:], start=True, stop=True)
            htmp = hp.tile([128, HID], F32, tag="htmp")
            nc.vector.tensor_add(htmp[:], h_ps[:], b2_sb[:])
            nc.scalar.activation(h2_all[:, nb * HID:(nb + 1) * HID], htmp[:],
                                 AF.Tanh)

        pool_ps = []    # single PSUM bank: [128, NSEGCH*(HID+1)] f32

        def group_epilogue(g):
            """Norms, logmap scale, and pooling matmuls for group g's four
            blocks — runs inline so the tail overlaps layer-2 gathers."""
            g0, g1 = g * GRP, (g + 1) * GRP
            for nb in range(g0, g1):
                h2b = h2_all[:, nb * HID:(nb + 1) * HID]
                sq = htp.tile([128, HID], F32, tag="sq")
                nc.vector.tensor_mul(sq[:], h2b, h2b)
                nc.vector.tensor_reduce(norms2[:, nb:nb + 1], sq[:],
                                        mybir.AxisListType.X, ALU.add)
            s_ = slice(g0, g1)
            # norm = sqrt(max(ss, MIN_SS)); nclip = min(norm, MAXNORM)
            nc.vector.tensor_scalar_max(na[:, s_], norms2[:, s_], MIN_SS)
            nc.scalar.activation(nb_t[:, s_], na[:, s_], AF.Sqrt)
            nc.vector.tensor_scalar_min(na[:, s_], nb_t[:, s_], MAXNORM)
            # artanh(nclip) = 0.5*ln((1+n)/(1-n)); scale = artanh/norm
            nc.vector.tensor_scalar(one_m[:, s_], na[:, s_], -1.0, 1.0,
                                    ALU.mult, ALU.add)
            nc.vector.tensor_scalar_add(one_p[:, s_], na[:, s_], 1.0)
            nc.vector.reciprocal(rcp[:, s_], one_m[:, s_])
            nc.vector.tensor_mul(rat[:, s_], one_p[:, s_], rcp[:, s_])
            nc.scalar.activation(lg[:, s_], rat[:, s_], AF.Ln)
            nc.vector.tensor_scalar_mul(lg[:, s_], lg[:, s_], 0.5)
            nc.vector.reciprocal(rcpn[:, s_], nb_t[:, s_])
            nc.vector.tensor_mul(scale[:, s_], lg[:, s_], rcpn[:, s_])
            for nb in range(g0, g1):
                h2b = h2_all[:, nb * HID:(nb + 1) * HID]
                ht = htp.tile([128, HID + 1], BF16, tag="ht")
                nc.vector.tensor_scalar(ht[:, :HID], h2b, scale[:, nb:nb + 1],
                                        None, ALU.mult)
                nc.vector.memset(ht[:, HID:HID + 1], 1.0)
                for sc in range(cfg.NSEGCH):
                    sg = sp.tile([128, 128], BF16, tag="sg")
                    nc.vector.tensor_scalar(
                        sg[:], iotaseg[:, sc * 128:(sc + 1) * 128],
                        segid[:, nb:nb + 1], None, ALU.is_equal)
                    # one shared bank: only the first matmul clears it
                    nc.tensor.matmul(
                        pool_ps[0][:, sc * (HID + 1):(sc + 1) * (HID + 1)],
                        sg[:], ht[:],
                        start=(nb == 0 and sc == 0),
                        stop=(nb == cfg.NBLK - 1))

        # ---------------- layer 1 (+ split AllGather issue) ----------------
        xtabs = {k: x_d[k * cfg.CH:(k + 1) * cfg.CH, :]
                 for k in range(cfg.NCHUNK)}
        spmm_pass(1, sched1, list(range(cfg.NCHUNK)), xtabs, idx1_d, slot1_d,
                  IN, l1_block, after_group=ag_hook)

        # ---------------- layer 2, pass 1 (local + half A) ----------------
        t2a = {0: h1_shard.ap(),
               1: h1_ha[0:cfg.CH, :],
               2: h1_ha[cfg.CH:2 * cfg.CH, :]}
        spmm_pass(2, sched2, list(L2P1), t2a, idx2_d, slot2_d, HID,
                  flush_block)

        # ---------------- layer 2, pass 2 (half B + combine + epilogue) ----
        ctx_l1.close()
        ps_pool = ctx.enter_context(
            tc.tile_pool(name="ps_pool", bufs=1, space="PSUM"))
        pool_ps.append(ps_pool.tile([128, cfg.NSEGCH * (HID + 1)], F32,
                                    tag="pool", name="pool_all"))
        t2b = {3: h1_hb[0:cfg.CH, :],
               4: h1_hb[cfg.CH:2 * cfg.CH, :]}
        spmm_pass(2, sched2, list(L2P2), t2b, idx2_d, slot2_d, HID,
                  l2_final, after_group=group_epilogue)

        for sc in range(cfg.NSEGCH):
            po = htp.tile([128, HID + 1], F32, tag="po")
            nc.vector.tensor_copy(
                po[:], pool_ps[0][:, sc * (HID + 1):(sc + 1) * (HID + 1)])
            nc.sync.dma_start(out_d[sc * 128:(sc + 1) * 128, :], po[:])

    nc.compile()
    return nc


def host_inputs(cfg, x, seg_ids, W1, b1, W2, b2, per_core, gpos):
    """Per-core in_maps for run_bass_kernel_spmd."""
    N, IN, HID = cfg.N, cfg.IN, cfg.HID
    x_bf16 = np.ascontiguousarray(x.astype(ml_dtypes.bfloat16))
    iota128 = np.tile(np.arange(128, dtype=np.float32), (128, 1)).astype(ml_dtypes.bfloat16)
    iotaseg = np.tile(np.arange(cfg.NSEGCH * 128, dtype=np.float32), (128, 1))
    ident = np.eye(128, dtype=np.float32).astype(ml_dtypes.bfloat16)
    w1 = np.ascontiguousarray(W1.astype(ml_dtypes.bfloat16))
    w2 = np.ascontiguousarray(W2.astype(ml_dtypes.bfloat16))
    b1r = np.tile(np.asarray(b1, np.float32), (128, 1))
    b2r = np.tile(np.asarray(b2, np.float32), (128, 1))
    seg_pos = np.empty(cfg.N, np.float32)
    seg_pos[gpos] = np.asarray(seg_ids, np.float32)   # seg id by position
    maps = []
    for c in range(cfg.NC):
        segc = seg_pos[c * cfg.SHARD:(c + 1) * cfg.SHARD].reshape(cfg.NBLK, 128).T
        maps.append({
            "x_bf16": x_bf16,
            "idx16_l1": per_core[c]["idx16_l1"],
            "idx16_l2": per_core[c]["idx16_l2"],
            "dstslot1": per_core[c]["dstslot1"],
            "dstslot2": per_core[c]["dstslot2"],
            "segid": np.ascontiguousarray(segc),
            "iota128": iota128,
            "iota_seg": np.ascontiguousarray(iotaseg.astype(np.float32)),
            "ident": ident,
            "W1": w1,
            "W2": w2,
            "b1rep": b1r,
            "b2rep": b2r,
        })
    return maps


def host_epilogue(cfg, partials, batch_size, max_comments):
    """partials: list of per-core [NSEGCH*128, HID+1] f32."""
    acc = np.zeros_like(partials[0], dtype=np.float64)
    for p in partials:
        acc += p.astype(np.float64)
    acc = acc.astype(np.float32)
    nseg = cfg.NSEG
    sums = acc[:nseg, :cfg.HID]
    counts = acc[:nseg, cfg.HID]
    agg = sums / np.maximum(counts, 1.0)[:, None]
    # expmap0 then proj
    ss = np.maximum(np.sum(agg * agg, axis=1), MIN_SS).astype(np.float32)
    norm = np.sqrt(ss)
    y = agg * (np.tanh(norm) / norm)[:, None]
    ssy = np.maximum(np.sum(y * y, axis=1), MIN_SS).astype(np.float32)
    ny = np.sqrt(ssy)
    f = np.where(ny > MAXNORM, MAXNORM / ny, 1.0).astype(np.float32)
    y = y * f[:, None]
    return y.reshape(int(batch_size), int(max_comments), cfg.HID)


# ====================================================================
# Harness entry point: kernel(**inputs) -> np.ndarray
# ====================================================================

_CACHE = {}


def kernel(x, src, dst, seg_ids, W1, b1, W2, b2, batch_size, max_comments):
    """Full-input GNN ComEnc kernel on 8 Trainium2 NeuronCores.

    Accepts the unsharded inputs of reference.setup_inputs() and returns
    the full (batch, max_comments, HID) float32 output.
    """
    from concourse.bass_utils import run_bass_kernel_spmd

    x = np.asarray(x, dtype=np.float32)
    src = np.asarray(src).astype(np.int64)
    dst = np.asarray(dst).astype(np.int64)
    seg_ids = np.asarray(seg_ids).astype(np.int64)
    W1 = np.asarray(W1, dtype=np.float32)
    b1 = np.asarray(b1, dtype=np.float32)
    W2 = np.asarray(W2, dtype=np.float32)
    b2 = np.asarray(b2, dtype=np.float32)
    bs = int(np.asarray(batch_size))
    mc = int(np.asarray(max_comments))

    n_nodes, in_dim = x.shape
    hid = W1.shape[1]
    nseg = bs * mc
    n_cores = 8

    cfg = Cfg(n_nodes, in_dim, hid, nseg, n_cores)
    ntiles1, ntiles2, per_core, gpos = host_prep(cfg, src, dst)

    key = (n_nodes, in_dim, hid, nseg, ntiles1.tobytes(), ntiles2.tobytes())
    if key in _CACHE:
        nc = _CACHE[key]
    else:
        nc = build(cfg, ntiles1, ntiles2)
        _CACHE.clear()
        _CACHE[key] = nc

    maps = host_inputs(cfg, x, seg_ids, W1, b1, W2, b2, per_core, gpos)
    res = run_bass_kernel_spmd(nc, maps, core_ids=list(range(n_cores)))
    partials = [r["pooled"] for r in res.results]
    out = host_epilogue(cfg, partials, bs, mc)
    return np.ascontiguousarray(out.astype(np.float32))
